# revision 63
# baseline (speedup 1.0000x reference)
"""Trainium2 Bass kernel for a BasicTransformerBlock (self-attn + cross-attn + GEGLU FF).

Sharding: data-parallel over the batch axis - 8 batch elements onto 8 NeuronCores,
same SPMD program, no collectives.

v3 design (v2 + act-table + scheduling/balance work; 247us -> 233us):
- Heavy matmuls run fp8e4 (TRN e4m3) with DoubleRow perf mode (0.5 cyc/row, K=256
  per instruction). Weights are power-of-2-scaled + converted to fp8 on the HOST
  and DMA'd straight into SBUF; QKV1 weights DMA first so LN1->projections are
  never input-starved.
- Residual stream stays transposed f32r xT[d, s]. LayerNorm gain g folds into
  the weights host-side. 1/std folds ONCE into zv1 = (x-mu)*SX*rstd (column
  scaling commutes through feature-mixing matmuls), so q/k/v all project from
  zv1 and their PSUM->SBUF evacuations are pure scales on the (then-idle) ACT
  engine as Identity ops. x psum->sbuf restage also runs on ACT at load time.
- Softmax: scores land in 2-bank PSUM pairs, one ACT exp per pair writing fp8
  probs 16*e^s in DoubleRow [p,2,n] layout; denominators via ones-lhsT
  DoubleRow matmuls. aT stays head-dim-major [64, NH, S] (walrus rejects any
  nonzero matmul psum dst partition); normalization = one DVE reciprocal +
  Pool partition-broadcasts + one fused stt per (c, hf).
- Activation tables: bacc's auto-pass is fed a doctored table list where only
  natural_log_exp_and_others (Ln/Exp/Identity) and gelu_and_others (Gelu/Copy)
  are matchable, collapsing 19 alternating table loads (~24us ACT) to 2-3.
  Gelus run as contiguous per-chunk blocks; ACT copies use Identity in the
  Ln/Exp phase and Copy in the gelu phase.
- Scales: activations x16, probs x16, attention out x64, per-weight 2^s with
  s = floor(log2(200/absmax)).
- Software pipelining: attention tails deferred one hf; encoder K/V block
  emitted inside the self-attention window; GEGLU chunk 0 emitted before
  cross-attention chunk 1 so the FF-c0 DVE/PE tail overlaps the thin
  cross-attn latency chains; FF psum g-tiles alternate between the psP slot
  and the stats slot (double-buffering); LN xsq and zv multiplies split
  across DVE and Pool. The LN mu-broadcast psum and the mo==0 / even-tp
  projection psums rotate through the 1-buf stats slot instead of the score
  slots, so attention score matmuls (which feed the ACT-critical exp stream)
  are never starved of psum during the LN2/LN3/v-proj windows.
"""
import sys

sys.path.insert(0, "/opt/trn_rl_repo")

import math
from contextlib import ExitStack

import ml_dtypes
import numpy as np

import concourse.bass as bass
import concourse.mybir as mybir
import concourse.tile as tile
from concourse import bacc
from concourse.bass_utils import run_bass_kernel_spmd
from concourse.masks import make_identity

F32 = mybir.dt.float32
F32R = mybir.dt.float32r
FP8 = mybir.dt.float8e4
AF = mybir.ActivationFunctionType
ALU = mybir.AluOpType
PM = mybir.MatmulPerfMode

B = 8
S = 1024          # tokens
D = 512           # model dim
SK2 = 77          # cross-attention source length
DE = 768          # encoder dim
FF = 2048         # GEGLU inner dim (per half)
NH = 8            # heads
DH = 64           # head dim
DHP = 80          # padded head slot in v tiles (16B-aligned pair strides);
                  # col 64 holds the ones column that folds the softmax
                  # denominator into the pv matmul (M=65, den = pv[64])
SCALE = DH ** -0.5
EPS = 1e-5
P = 128
NC = 512          # token chunk (one psum bank of fp32)
ST = S // P       # 8 token tiles
FT = D // P       # 4 feature tiles
CH = S // NC      # 2 token chunks
KE = DE // P      # 6 encoder feature tiles
NI = FF // P      # 16 FF inner tiles

SX = 16.0         # activation scale (z, enc, q, k, v)
SEXP = 16.0       # probs scale
SA = 64.0         # attention-output scale
LN_SEXP = math.log(SEXP)

WNAMES = ["wq1", "wk1", "wv1", "wo1", "wq2", "wk2", "wv2", "wo2", "wg", "wf"]


def build(nc: bass.Bass, ws: dict, flags: dict, dbg=False):
    """ws: name -> log2 weight scale. flags: name -> bias vector is nonzero."""
    x = nc.dram_tensor("x", [S, D], F32, kind="ExternalInput")
    enc = nc.dram_tensor("enc", [SK2, DE], F32, kind="ExternalInput")
    w_in = {}
    for name, shape in [
        ("wq1", [D, D]), ("wk1", [D, D]), ("wv1", [D, D]), ("wo1", [D, D]),
        ("wq2", [D, D]), ("wk2", [DE, D]), ("wv2", [DE, D]), ("wo2", [D, D]),
        ("wg", [D, 2 * FF]), ("wf", [FF, D]),
    ]:
        w_in[name] = nc.dram_tensor(name, shape, FP8, kind="ExternalInput")
    vec_in = {}
    for name, n in [("bq1", D), ("bk1", D), ("bv1", D), ("bq2", D),
                    ("bo1", D), ("bo2", D), ("bf", D),
                    ("bgu", FF), ("bgg", FF)]:
        vec_in[name] = nc.dram_tensor(name, [n], F32, kind="ExternalInput")
    out = nc.dram_tensor("out", [S, D], F32, kind="ExternalOutput")
    dbg_out = {}
    if dbg:
        for name, shape, dt in [
            ("d_qT1", [P, FT, S], FP8), ("d_kT1", [P, FT, S], FP8),
            ("d_v1", [P, ST // 2, 2, NH, DH], FP8),
            ("d_aT1", [DH, NH, S], FP8), ("d_xT1", [P, FT, S], F32R),
            ("d_qT2", [P, FT, S], FP8), ("d_aT2", [DH, NH, S], FP8),
            ("d_xT2", [P, FT, S], F32R), ("d_z3", [P, FT, S], FP8),
            ("d_fft0", [P, 2, NC], FP8), ("d_kT2", [P, FT, 80], FP8),
            ("d_rc", [1, 2, NC], F32), ("d_bc", [DH, 2, NC], F32),
            ("d_v2", [SK2, NH, DH], FP8), ("d_encT", [P, KE, 80], FP8),
        ]:
            dbg_out[name] = nc.dram_tensor(name, shape, dt,
                                           kind="ExternalOutput")

    cw = {n: 2.0 ** -ws[n] for n in WNAMES}   # descale constants

    with tile.TileContext(nc) as tc, ExitStack() as es:
        const = es.enter_context(tc.tile_pool(name="const", bufs=1))
        resid = es.enter_context(tc.tile_pool(name="resid", bufs=2))
        zp = es.enter_context(tc.tile_pool(name="zp", bufs=2))
        att = es.enter_context(tc.tile_pool(name="att", bufs=1))
        wp = es.enter_context(tc.tile_pool(name="wp", bufs=1))
        stage = es.enter_context(tc.tile_pool(name="stage", bufs=2))
        t3p = es.enter_context(tc.tile_pool(name="t3p", bufs=1))
        rowp = es.enter_context(tc.tile_pool(name="rowp", bufs=1))
        bcp = es.enter_context(tc.tile_pool(name="bcp", bufs=3))
        exp_p = es.enter_context(tc.tile_pool(name="exp", bufs=3))
        gelp = es.enter_context(tc.tile_pool(name="gelp", bufs=2))
        fftp = es.enter_context(tc.tile_pool(name="fftp", bufs=8))
        psA = es.enter_context(tc.tile_pool(name="psA", bufs=2, space="PSUM"))
        psP = es.enter_context(tc.tile_pool(name="psP", bufs=1, space="PSUM"))

        # ---- constants ----
        ident = const.tile([P, P], F32, tag="ident")
        make_identity(nc, ident[:])
        ones_f = const.tile([P, P], F32, tag="ones_f")
        nc.vector.memset(ones_f[:], 1.0)
        ones128 = const.tile([P, 1], F32R, tag="o128")  # stats lhsT (K=128,M=1)
        nc.vector.tensor_copy(ones128[:], ones_f[:, 0:1])
        ones1x = const.tile([1, P], F32R, tag="o1x")    # bcast lhsT (K=1)
        nc.vector.tensor_copy(ones1x[:], ones_f[0:1, :])
        ones_row = None
        if any(flags.values()):
            ones_row_f = const.tile([1, NC], F32, tag="orowf")
            nc.vector.memset(ones_row_f[:], 1.0)
            ones_row = const.tile([1, NC], F32R, tag="orow")  # rank-1 bias rhs
            nc.vector.tensor_copy(ones_row[:], ones_row_f[:])
        ones8 = const.tile([P, 2, 16], FP8, tag="o8")   # self denominator lhsT
        nc.vector.memset(ones8[:], 1.0)                 # (16B-aligned pair stride)
        ones2 = const.tile([SK2, 1], FP8, tag="o2")     # cross denominator lhsT
        nc.vector.memset(ones2[:], 1.0)
        lnsc = const.tile([P, 1], F32, tag="lnsc")      # exp bias = ln(SEXP)
        nc.vector.memset(lnsc[:], LN_SEXP)
        eps_t = const.tile([1, 1], F32, tag="eps")
        nc.vector.memset(eps_t[:], EPS)
        # explicit activation-table management: one Ln+Exp table for all of
        # layernorm + softmax, a single switch to the gelu table for the FF
        # tail (Copy is in both tables, so ACT evacuations never force a
        # switch). bacc's auto-insertion pass is disabled in _get_nc — it
        # maps Ln->natural_log and Exp->exp_and_others and thrashes 15+
        # table loads (~1.3us ACT each) across the LN/softmax interleave.
        try:
            from concourse.hw_specs import get_activation_tables
            _t = list(get_activation_tables(nc.m.arch))
            TAB_LNEXP = _t.index("natural_log_exp_and_others")
            TAB_GELU = _t.index("gelu_and_others")
        except Exception:
            TAB_LNEXP, TAB_GELU = 6, 10

        def load_table(tid):
            if tid is not None:
                nc.scalar.add_instruction(mybir.InstLoadActFuncSet(
                    name=nc.get_next_instruction_name(), ins=[], outs=[],
                    act_func_set_id=tid))

        warm = const.tile([1, 1], F32, tag="warm")
        nc.scalar.activation(warm[:], eps_t[:], AF.Ln)   # pull act-table load
        nc.scalar.activation(warm[:], warm[:], AF.Exp)   # into the DMA window

        # ---- weights: DMA fp8 straight into SBUF ----
        def load_w(name, kouter):
            dram = w_in[name]
            wr = wp.tile([P, kouter, dram.shape[1]], FP8, tag=f"{name}r")
            nc.sync.dma_start(wr[:], dram.rearrange("(ko ki) n -> ki ko n", ki=P))
            return wr

        bgg_c = const.tile([P, NI], F32, tag="bgg_c")
        if flags["bgg"]:
            nc.sync.dma_start(bgg_c[:],
                              vec_in["bgg"].rearrange("(o p) -> p o", p=P))

        def bias_row(name):
            """[1, N] f32r row of host-scaled bias, for rank-1 lhsT slices."""
            n = vec_in[name].shape[0]
            tf = const.tile([1, n], F32, tag=f"{name}_f")
            nc.sync.dma_start(tf[:], vec_in[name].rearrange("(o n) -> o n", o=1))
            tr = const.tile([1, n], F32R, tag=f"{name}_r")
            nc.vector.tensor_copy(tr[:], tf[:])
            return tr

        brow = {name: bias_row(name)
                for name in ("bq1", "bk1", "bv1", "bq2", "bo1", "bo2", "bf",
                             "bgu") if flags[name]}

        # ---- load x, transpose into xT [128, FT, S] (f32r) ----
        # x and enc DMAs go first so the transposes are never input-starved;
        # weight DMAs queue behind them and overlap LN1/QKV1 compute.
        xT = resid.tile([P, FT, S], F32R, tag="x")
        x_stage = []
        for st in range(ST):
            xr = stage.tile([P, D], F32, tag="x_raw", bufs=3)
            nc.sync.dma_start(xr[:], x[P * st:P * (st + 1), :])
            x_stage.append(xr)
        def load_wo(name):
            dram = w_in[name]
            wr = wp.tile([DH, NH, dram.shape[1]], FP8, tag=f"{name}r")
            nc.sync.dma_start(wr[:], dram.rearrange("(h k) n -> k h n", k=DH))
            return wr

        wq1r = load_w("wq1", FT)
        wk1r = load_w("wk1", FT)
        wv1r = load_w("wv1", FT)
        enc_raw = const.tile([SK2, DE], F32, tag="enc_raw")
        nc.sync.dma_start(enc_raw[:], enc[:, :])
        wo1r = load_wo("wo1")
        wq2r = load_w("wq2", FT)
        wk2r = load_w("wk2", KE)
        wv2r = load_w("wv2", KE)
        wo2r = load_wo("wo2")
        wgr = load_w("wg", FT)
        wfr = load_w("wf", NI)
        def load_x_pair(tp):
            pt = psA.tile([P, 2, NC], F32, tag="pA")
            for half in range(2):
                st = 2 * tp + half
                xr = x_stage[st]
                for ft in range(FT):
                    nc.tensor.transpose(pt[:, half, P * ft:P * (ft + 1)],
                                        xr[:, P * ft:P * (ft + 1)], ident[:])
            # psum->xT restage on ACT (idle during the load phase;
            # Identity is in the Ln/Exp table so no act-table switch)
            nc.scalar.activation(
                xT[:, :, 2 * P * tp:2 * P * (tp + 1)].rearrange(
                    "p f (a q) -> p a f q", a=2),
                pt[:].rearrange("p a (f q) -> p a f q", f=FT), AF.Identity)

        # ---- enc -> encT [128, KE, 80] fp8 (x16), cols 77:80 zero-padded
        # (pair slices need 16B-aligned outer stride for dual-fp8 ldweights).
        # Emitted later, during ACT-bound self-attention: only cross-attn
        # consumes these, and their DVE copies would otherwise delay LN1. ----
        SK2P = 80
        SK2A = 80   # padded so [hp, hf, :] slices stay 4B-aligned
        encT = att.tile([P, KE, SK2P], FP8, tag="encT")
        kT2 = att.tile([P, FT, SK2A], FP8, tag="kT2")
        v2sb = att.tile([SK2, NH, DH], FP8, tag="v2sb")

        def emit_enc_block():
            pass

        def _enc_block():
            nc.vector.memset(encT[:, :, SK2:SK2P], 0.0)
            for kp in range(KE // 2):
                pt = psA.tile([P, 2, NC], F32, tag="pA")
                for half in range(2):
                    ke = 2 * kp + half
                    nc.tensor.transpose(pt[:, half, 0:SK2],
                                        enc_raw[:, P * ke:P * (ke + 1)],
                                        ident[0:SK2, 0:SK2])
                nc.vector.tensor_scalar_mul(
                    encT[:, 2 * kp:2 * kp + 2, 0:SK2], pt[:, :, 0:SK2], SX)
            for mp in range(FT // 2):
                pt = psA.tile([P, 2, NC], F32, tag="pA")
                for half in range(2):
                    mo = 2 * mp + half
                    for t in range(KE // 2):
                        nc.tensor.matmul(pt[:, half, 0:SK2P],
                                         wk2r[:, 2 * t:2 * t + 2,
                                              P * mo:P * (mo + 1)],
                                         encT[:, 2 * t:2 * t + 2, :],
                                         start=(t == 0),
                                         stop=(t == KE // 2 - 1),
                                         perf_mode=PM.DoubleRow)
                nc.vector.tensor_scalar_mul(kT2[:, 2 * mp:2 * mp + 2, 0:SK2],
                                            pt[:, :, 0:SK2], cw["wk2"])
            ptv = psA.tile([P, 2, NC], F32, tag="pA", name="ptv2")
            for t in range(KE // 2):
                nc.tensor.matmul(ptv[0:SK2P, 0, :],
                                 encT[:, 2 * t:2 * t + 2, :],
                                 wv2r[:, 2 * t:2 * t + 2, :],
                                 start=(t == 0), stop=(t == KE // 2 - 1),
                                 perf_mode=PM.DoubleRow)
            nc.vector.tensor_scalar_mul(
                v2sb[:].rearrange("s h d -> s (h d)"), ptv[0:SK2, 0, :],
                cw["wv2"])

        # ---- LayerNorm: stats + rows; z via gpsimd (all-SBUF) ----
        def ln_sums(src, c, tagpfx):
            cs = slice(NC * c, NC * (c + 1))
            sum_ps = psA.tile([1, 2, NC], F32, tag="den", bufs=1,
                              name=f"st_{tagpfx}{c}")
            for ft in range(FT):
                nc.tensor.matmul(sum_ps[:, 0, :], ones128[:], src[:, ft, cs],
                                 start=(ft == 0), stop=(ft == FT - 1))
            for ft in range(FT):
                xsq = stage.tile([P, NC], F32R, tag="xsq")
                sq_eng = nc.vector if (tagpfx == "l1" or ft < 2) else nc.gpsimd
                sq_eng.tensor_tensor(xsq[:], src[:, ft, cs], src[:, ft, cs],
                                     ALU.mult)
                nc.tensor.matmul(sum_ps[:, 1, :], ones128[:], xsq[:],
                                 start=(ft == 0), stop=(ft == FT - 1),
                                 skip_group_check=True)
            return sum_ps

        def ln_stats(src, c, tagpfx, rstd_dtype=F32, sum_ps=None):
            """-> (mu16_sb [P,NC], rstd row, lnv row) for chunk c of src."""
            if sum_ps is None:
                sum_ps = ln_sums(src, c, tagpfx)
            mu = rowp.tile([1, NC], F32, tag="mu", bufs=2)
            nc.vector.tensor_scalar_mul(mu[:], sum_ps[:, 0, :], 1.0 / D)
            mu16 = rowp.tile([1, NC], F32R, tag="mu16")
            nc.vector.tensor_scalar_mul(mu16[:], mu[:], SX)
            musq = rowp.tile([1, NC], F32, tag="musq")
            nc.gpsimd.tensor_tensor(musq[:], mu[:], mu[:], ALU.mult)
            var = rowp.tile([1, NC], F32, tag="var", bufs=2)
            nc.vector.scalar_tensor_tensor(var[:], sum_ps[:, 1, :], 1.0 / D,
                                           musq[:], op0=ALU.mult,
                                           op1=ALU.subtract)
            lnv = rowp.tile([1, NC], F32, tag="lnv", bufs=2)
            nc.scalar.activation(lnv[:], var[:], AF.Ln, bias=eps_t[:])
            rstd = rowp.tile([1, NC], rstd_dtype, tag="rstd", bufs=2)
            nc.scalar.activation(rstd[:], lnv[:], AF.Exp, scale=-0.5)
            # rotate through the stats slot rather than a score slot so
            # attention score matmuls aren't starved during LN2/LN3
            mbt = psA.tile([P, 2, NC], F32, tag="den", bufs=1,
                           name=f"mub_{tagpfx}{c}")
            nc.tensor.matmul(mbt[:, 0, :], ones1x[:], mu16[:], start=True,
                             stop=True)
            return mbt[:, 0:1, :], rstd, lnv

        def z_from(src, mu16_b, c, zt):
            cs = slice(NC * c, NC * (c + 1))
            for mo in range(0, FT, 2):
                nc.vector.scalar_tensor_tensor(
                    zt[:, mo:mo + 2, cs], src[:, mo:mo + 2, cs], SX,
                    mu16_b.broadcast_to([P, 2, NC]),
                    op0=ALU.mult, op1=ALU.subtract)

        def sd_row_from(lnv):
            sd = rowp.tile([1, NC], F32R, tag="sd", bufs=2)
            nc.scalar.activation(sd[:], lnv[:], AF.Exp, scale=0.5)
            return sd

        # ---- projection helpers ----
        def project_pair(w_r, wname, zt, c, mo, dst, rstd_sb=None, bname=None,
                         sd_row=None, evac="act"):
            """dst[:, mo:mo+2, cs] = (W.T@z * cw) [* rstd] (+ bias via rank-1).
            evac: 'act' = ACT Identity (pure scale; zt must carry rstd
            already), 'dve' = DVE tensor_scalar (pure scale), 'dve_rstd' =
            DVE stt with per-token rstd (zt un-rstd'd)."""
            cs = slice(NC * c, NC * (c + 1))
            pp = psA.tile([P, 2, NC], F32, tag="den" if mo == 0 else "pA",
                          bufs=1 if mo == 0 else 2)
            for half in range(2):
                m = mo + half
                for t in range(FT // 2):
                    nc.tensor.matmul(pp[:, half, :],
                                     w_r[:, 2 * t:2 * t + 2, P * m:P * (m + 1)],
                                     zt[:, 2 * t:2 * t + 2, cs],
                                     start=(t == 0), stop=(t == FT // 2 - 1),
                                     perf_mode=PM.DoubleRow)
                if bname is not None and bname in brow:
                    nc.tensor.matmul(pp[:, half, :],
                                     brow[bname][:, P * m:P * (m + 1)],
                                     sd_row[:], start=False, stop=True,
                                     skip_group_check=True)
            if evac == "act":
                nc.scalar.activation(dst[:, mo:mo + 2, cs], pp[:],
                                     AF.Identity, scale=cw[wname])
            elif evac == "dve":
                nc.vector.tensor_scalar_mul(dst[:, mo:mo + 2, cs], pp[:],
                                            cw[wname])
            else:
                nc.vector.scalar_tensor_tensor(
                    dst[:, mo:mo + 2, cs], pp[:], cw[wname],
                    rstd_sb[:, None, :].broadcast_to([P, 2, NC]),
                    op0=ALU.mult, op1=ALU.mult)

        def out_proj_pair(w_r, wname, aT, c, mo, src, dst, post, bname=None):
            """dst[:, mo:mo+2, cs] = src + (W.T@a * cw * post) [+ bias].
            aT is head-dim-major [DH, NH, S]; contraction runs over head
            pairs (K = 2*DH per DoubleRow matmul)."""
            cs = slice(NC * c, NC * (c + 1))
            pp = psA.tile([P, 2, NC], F32, tag="den" if mo == 0 else "pA",
                          bufs=1 if mo == 0 else 2)
            for half in range(2):
                m = mo + half
                for t in range(NH // 2):
                    nc.tensor.matmul(pp[:, half, :],
                                     w_r[:, 2 * t:2 * t + 2, P * m:P * (m + 1)],
                                     aT[:, 2 * t:2 * t + 2, cs],
                                     start=(t == 0), stop=(t == NH // 2 - 1),
                                     perf_mode=PM.DoubleRow)
                if bname is not None and bname in brow:
                    nc.tensor.matmul(pp[:, half, :],
                                     brow[bname][:, P * m:P * (m + 1)],
                                     ones_row[:], start=False, stop=True,
                                     skip_group_check=True)
            nc.vector.scalar_tensor_tensor(
                dst[:, mo:mo + 2, cs], pp[:], cw[wname] * post,
                src[:, mo:mo + 2, cs], op0=ALU.mult, op1=ALU.add)

        # ---- attention (self) for one chunk ----
        # rc_act: cross-attn tails land in the (DVE-bound, ACT-idle) FF
        # window - compute 1/den as Exp(-Ln(den)) on ACT there instead of
        # a DVE reciprocal. Self-attn tails run while ACT is exp-saturated,
        # so they keep the DVE reciprocal.
        def attn_tail(aT, c, hf, pv, den):
            cs = slice(NC * c, NC * (c + 1))
            rc = rowp.tile([1, 2, NC], F32, tag="rc", bufs=2)
            nc.vector.reciprocal(rc[:], den[:])
            bc = bcp.tile([DH, 2, NC], F32, tag="bc", bufs=3)
            nc.gpsimd.partition_broadcast(bc[:, 0, :], rc[:, 0, :])
            nc.gpsimd.partition_broadcast(bc[:, 1, :], rc[:, 1, :])
            nc.vector.scalar_tensor_tensor(
                aT[:, 2 * hf:2 * hf + 2, cs], pv[:], SA / SEXP,
                bc[:], op0=ALU.mult, op1=ALU.mult)

        pend = []   # software-pipelined attention tails across calls

        def flush_tail():
            if pend:
                attn_tail(*pend.pop())

        def attn_self(qT, kT, v1sb, aT, c, hfs, mid=None):
            cs = slice(NC * c, NC * (c + 1))
            for hf in hfs:
                exs = []
                for par in range(2):
                    hp = slice(DH * par, DH * par + DH)
                    for j in range(ST // 2):
                        if mid is not None and hf == hfs[0]:
                            if par == 0 and j == 2:
                                mid(0)
                            elif par == 1 and j == 0:
                                mid(1)
                        sc = psA.tile([P, 2, NC], F32, tag="pA")
                        for half in range(2):
                            sk = 2 * j + half
                            nc.tensor.matmul(
                                sc[:, half, :],
                                kT[hp, hf, P * sk:P * (sk + 1)],
                                qT[hp, hf, cs], start=True, stop=True)
                        ex = exp_p.tile([P, 2, NC], FP8, tag="ex", bufs=6)
                        nc.scalar.activation(ex[:], sc[:], AF.Exp,
                                             scale=SCALE / (SX * SX),
                                             bias=lnsc[:])
                        exs.append(ex)
                flush_tail()
                pv = psP.tile([DH, 2, NC], F32, tag="pvp",
                              name=f"pv1_{c}_{hf}")
                den = psA.tile([1, 2, NC], F32, tag="den", bufs=1,
                               name=f"den1_{c}_{hf}")
                for par in range(2):
                    h = 2 * hf + par
                    for j in range(ST // 2):
                        ex = exs[par * (ST // 2) + j]
                        nc.tensor.matmul(pv[:, par, :],
                                         v1sb[:, j, :, h, :], ex[:],
                                         start=(j == 0),
                                         stop=(j == ST // 2 - 1),
                                         perf_mode=PM.DoubleRow,
                                         skip_group_check=True)
                        nc.tensor.matmul(den[:, par, :], ones8[:, :, 0:1], ex[:],
                                         start=(j == 0), stop=(j == ST // 2 - 1),
                                         perf_mode=PM.DoubleRow,
                                         skip_group_check=True)
                pend.append((aT, c, hf, pv, den))

        # ---- attention (cross) for one chunk ----
        def attn_cross(qT, c, hfs):
            aT = lz("aT2", lambda: att.tile([DH, NH, S], FP8, tag="ka",
                                            bufs=1, name="aT2"))
            cs = slice(NC * c, NC * (c + 1))
            for hf in hfs:
                sc = psA.tile([P, 2, NC], F32, tag="pA")
                for par in range(2):
                    hp = slice(DH * par, DH * par + DH)
                    nc.tensor.matmul(sc[0:SK2, par, :], kT2[hp, hf, 0:SK2],
                                     qT[hp, hf, cs], start=True, stop=True)
                ex = exp_p.tile([SK2, 2, NC], FP8, tag="ex2c", bufs=2)
                nc.scalar.activation(ex[:], sc[0:SK2, :, :], AF.Exp,
                                     scale=SCALE / (SX * SX),
                                     bias=lnsc[0:SK2, :])
                flush_tail()
                pv = psP.tile([DH, 2, NC], F32, tag="pvp",
                              name=f"pv2_{c}_{hf}")
                den = psA.tile([1, 2, NC], F32, tag="den", bufs=1,
                               name=f"den2_{c}_{hf}")
                for par in range(2):
                    h = 2 * hf + par
                    nc.tensor.matmul(pv[:, par, :],
                                     v2sb[:, h, :], ex[:, par, :],
                                     start=True, stop=True,
                                     skip_group_check=True)
                    nc.tensor.matmul(den[:, par, :], ones2[:], ex[:, par, :],
                                     start=True, stop=True,
                                     skip_group_check=True)
                pend.append((aT, c, hf, pv, den))

        # ================= LN1 + QKV (both chunks) =================
        # rstd folds into zv1 once (column scaling commutes through the
        # feature-mixing projections), so q/k/v all project from zv1 and
        # their evacuations become pure scales (ACT Identity / DVE scale)
        zv1 = zp.tile([P, FT, S], FP8, tag="z")
        qT1 = att.tile([P, FT, S], FP8, tag="qt", bufs=1)
        kT1 = att.tile([P, FT, S], FP8, tag="ka", bufs=1)
        v1sb = att.tile([P, ST // 2, 2, NH, DH], FP8, tag="v1sb")
        rstd1_sb = {}
        sd1 = {}
        need_sd1 = flags["bq1"] or flags["bk1"] or flags["bv1"]
        def v_proj_pair(c, tp):
            pp = psA.tile([P, 2, NC], F32, tag="den" if tp % 2 == 0 else "pA",
                          bufs=1 if tp % 2 == 0 else 2)
            for half in range(2):
                stt = 2 * tp + half
                for t in range(FT // 2):
                    nc.tensor.matmul(
                        pp[:, half, :],
                        zv1[:, 2 * t:2 * t + 2, P * stt:P * (stt + 1)],
                        wv1r[:, 2 * t:2 * t + 2, :],
                        start=(t == 0), stop=(t == FT // 2 - 1),
                        perf_mode=PM.DoubleRow)
                if flags["bv1"]:
                    off = P * stt - NC * c
                    nc.tensor.matmul(
                        pp[:, half, :],
                        sd1[c][:, off:off + P], brow["bv1"][:],
                        start=False, stop=True, skip_group_check=True)
            nc.vector.tensor_scalar_mul(
                v1sb[:, tp, :, :, :].rearrange("p a h d -> p (a h d)"),
                pp[:].rearrange("p a n -> p (a n)"), cw["wv1"])

        for tp in range(ST // 2):
            load_x_pair(tp)
        mu16b1 = {}
        for c in range(CH):
            cs = slice(NC * c, NC * (c + 1))
            mu16_b, rstd, lnv = ln_stats(xT, c, "l1")
            mu16b1[c] = mu16_b
            rsb = bcp.tile([P, NC], F32, tag="rstdb", name=f"rstd1b_{c}")
            nc.gpsimd.partition_broadcast(rsb[:], rstd[:])
            rstd1_sb[c] = rsb
            if need_sd1:
                sd1[c] = sd_row_from(lnv)
            for mo in range(0, FT, 2):
                t1 = t3p.tile([P, 2, NC], F32, tag="t1", bufs=2)
                nc.vector.scalar_tensor_tensor(
                    t1[:], xT[:, mo:mo + 2, cs], SX,
                    mu16_b.broadcast_to([P, 2, NC]),
                    op0=ALU.mult, op1=ALU.subtract)
                zv_eng = nc.vector if mo == 0 else nc.gpsimd
                zv_eng.tensor_tensor(
                    zv1[:, mo:mo + 2, cs], t1[:],
                    rsb[:, None, :].broadcast_to([P, 2, NC]), ALU.mult)
        # K projections first (attention needs all keys), then q(c0)/v, then
        # q(c1) is deferred into the first attention call
        for c in range(CH):
            for mo in range(0, FT, 2):
                project_pair(wk1r, "wk1", zv1, c, mo, kT1, None,
                             "bk1", sd1.get(c), evac="act")
        for mo in range(0, FT, 2):
            project_pair(wq1r, "wq1", zv1, 0, mo, qT1, None,
                         "bq1", sd1.get(0), evac="act")

        def emit_v_projs(half):
            for tp in (2 * half, 2 * half + 1):
                v_proj_pair(tp // 2, tp)

        # ================= chunk-pipelined main pass =================
        # tiles are allocated lazily at first use so the resid/z/ka tag
        # rotations only reclaim slots whose previous tenant is dead
        aT1 = att.tile([DH, NH, S], FP8, tag="aT1")
        lazy = {}

        def lz(name, ctor):
            if name not in lazy:
                lazy[name] = ctor()
            return lazy[name]

        rstd2_sb = {}
        sd2 = {}
        fft2 = {0: [], 1: []}

        def pa1a(c):
            flush_tail()
            xT1 = lz("xT1", lambda: resid.tile([P, FT, S], F32R, tag="x",
                                               name="xT1"))
            for mo in range(0, FT, 2):
                out_proj_pair(wo1r, "wo1", aT1, c, mo, xT, xT1, 1.0 / SA, "bo1")

        def pa1b(c):
            xT1 = lazy["xT1"]
            mu16_sb, rstd, lnv = ln_stats(xT1, c, "l2")
            rsb = bcp.tile([P, NC], F32, tag="rstdb", name=f"rstd2b_{c}")
            nc.gpsimd.partition_broadcast(rsb[:], rstd[:])
            rstd2_sb[c] = rsb
            if flags["bq2"]:
                sd2[c] = sd_row_from(lnv)
            z2 = lz("z2", lambda: zp.tile([P, FT, S], FP8, tag="z",
                                          name="z2"))
            z_from(xT1, mu16_sb, c, z2)

        def pa1c(c):
            z2 = lazy["z2"]
            qT2 = lz("qT2", lambda: att.tile([P, FT, S], FP8, tag="qt",
                                             bufs=1, name="qT2"))
            for mo in range(0, FT, 2):
                project_pair(wq2r, "wq2", z2, c, mo, qT2, rstd2_sb[c],
                             "bq2", sd2.get(c), evac="dve_rstd")

        def pa2a1(c):
            flush_tail()
            xT1 = lazy["xT1"]
            aT2 = lazy["aT2"]
            xT2 = lz("xT2", lambda: resid.tile([P, FT, S], F32R, tag="x",
                                               name="xT2"))
            for mo in range(0, FT, 2):
                out_proj_pair(wo2r, "wo2", aT2, c, mo, xT1, xT2, 1.0 / SA, "bo2")

        def pa2a2(c):
            cs = slice(NC * c, NC * (c + 1))
            xT2 = lazy["xT2"]
            z3 = lz("z3", lambda: zp.tile([P, FT, S], FP8, tag="z", name="z3"))
            mu16_sb, rstd, lnv = ln_stats(xT2, c, "l3")
            r3b = bcp.tile([P, NC], F32, tag="rstdb", name=f"rstd3b_{c}")
            nc.gpsimd.partition_broadcast(r3b[:], rstd[:])
            for mo in range(0, FT, 2):
                t3 = t3p.tile([P, 2, NC], F32, tag="t1", bufs=2)
                nc.vector.scalar_tensor_tensor(
                    t3[:], xT2[:, mo:mo + 2, cs], SX,
                    mu16_sb.broadcast_to([P, 2, NC]),
                    op0=ALU.mult, op1=ALU.subtract)
                nc.gpsimd.tensor_tensor(
                    z3[:, mo:mo + 2, cs], t3[:],
                    r3b[:, None, :].broadcast_to([P, 2, NC]), ALU.mult)

        def pa2b(c, js):
            # GEGLU inner: g/u pair tiles -> gelu -> fft (fp8 DoubleRow layout)
            cs = slice(NC * c, NC * (c + 1))
            z3 = lazy["z3"]
            for j in js:
                if j % 2 == 0:
                    pg = psP.tile([P, 2, NC], F32, tag="pvp",
                                  name=f"pg_{c}_{j}")
                else:
                    pg = psA.tile([P, 2, NC], F32, tag="den", bufs=1,
                                  name=f"pg_{c}_{j}")
                pu = psA.tile([P, 2, NC], F32, tag="pA")
                for half in range(2):
                    i = 2 * j + half
                    for t in range(FT // 2):
                        nc.tensor.matmul(pg[:, half, :],
                                         wgr[:, 2 * t:2 * t + 2,
                                             FF + P * i:FF + P * (i + 1)],
                                         z3[:, 2 * t:2 * t + 2, cs],
                                         start=(t == 0), stop=(t == FT // 2 - 1),
                                         perf_mode=PM.DoubleRow)
                for half in range(2):
                    i = 2 * j + half
                    for t in range(FT // 2):
                        nc.tensor.matmul(pu[:, half, :],
                                         wgr[:, 2 * t:2 * t + 2,
                                             P * i:P * (i + 1)],
                                         z3[:, 2 * t:2 * t + 2, cs],
                                         start=(t == 0), stop=(t == FT // 2 - 1),
                                         perf_mode=PM.DoubleRow)
                    if flags["bgu"]:
                        nc.tensor.matmul(pu[:, half, :],
                                         brow["bgu"][:, P * i:P * (i + 1)],
                                         ones_row[:], start=False, stop=True,
                                         skip_group_check=True)
                gel = gelp.tile([P, 2, NC], FP8, tag="gel")
                if flags["bgg"]:
                    for half in range(2):
                        i = 2 * j + half
                        nc.scalar.activation(gel[:, half, :], pg[:, half, :],
                                             AF.Gelu, scale=cw["wg"] / SX,
                                             bias=bgg_c[:, i:i + 1])
                else:
                    nc.scalar.activation(gel[:], pg[:], AF.Gelu,
                                         scale=cw["wg"] / SX)
                ft8 = fftp.tile([P, 2, NC], FP8, tag="ft8")
                nc.vector.scalar_tensor_tensor(ft8[:], pu[:], cw["wg"], gel[:],
                                               op0=ALU.mult, op1=ALU.mult)
                fft2[c].append(ft8)

        def ff_out(c, mos=(0, 2)):
            cs = slice(NC * c, NC * (c + 1))
            xT2 = lazy["xT2"]
            xT3 = lz("xT3", lambda: resid.tile([P, FT, S], F32, tag="x",
                                               name="xT3"))
            for mo in mos:
                pp = psA.tile([P, 2, NC], F32, tag="pA")
                for half in range(2):
                    m = mo + half
                    for j in range(NI // 2):
                        nc.tensor.matmul(pp[:, half, :],
                                         wfr[:, 2 * j:2 * j + 2,
                                             P * m:P * (m + 1)],
                                         fft2[c][j][:],
                                         start=(j == 0), stop=(j == NI // 2 - 1),
                                         perf_mode=PM.DoubleRow)
                    if flags["bf"]:
                        nc.tensor.matmul(pp[:, half, :],
                                         brow["bf"][:, P * m:P * (m + 1)],
                                         ones_row[:], start=False, stop=True,
                                         skip_group_check=True)
                nc.vector.scalar_tensor_tensor(
                    xT3[:, mo:mo + 2, cs], pp[:], cw["wf"] / SX,
                    xT2[:, mo:mo + 2, cs], op0=ALU.mult, op1=ALU.add)

        def store(c, cfunc=AF.Copy):
            xT3 = lazy["xT3"]
            for tp in range(2 * c, 2 * c + 2):
                pp = psA.tile([P, 2, NC], F32, tag="pA")
                for half in range(2):
                    stt = 2 * tp + half
                    for ft in range(FT):
                        nc.tensor.transpose(
                            pp[:, half, P * ft:P * (ft + 1)],
                            xT3[:, ft, P * stt:P * (stt + 1)], ident[:])
                for half in range(2):
                    ot = stage.tile([P, D], F32, tag="ot", bufs=3)
                    # ACT evac; func chosen to match the active act table
                    # (Identity = Ln/Exp table, Copy = gelu table)
                    nc.scalar.activation(ot[:], pp[:, half, :], cfunc)
                    nc.sync.dma_start(
                        out[P * (2 * tp + half):P * (2 * tp + half + 1), :],
                        ot[:])

        def dump(name, tile_ap):
            if dbg:
                nc.sync.dma_start(dbg_out[name][...], tile_ap)

        attn_self(qT1, kT1, v1sb, aT1, 0, [0], mid=emit_v_projs)
        for mo in range(0, FT, 2):
            project_pair(wq1r, "wq1", zv1, 1, mo, qT1, None,
                         "bq1", sd1.get(1), evac="dve")
        attn_self(qT1, kT1, v1sb, aT1, 0, [1, 2, 3])
        _enc_block()
        attn_self(qT1, kT1, v1sb, aT1, 1, [0, 1])
        pa1a(0)
        attn_self(qT1, kT1, v1sb, aT1, 1, [2])
        pa1b(0)
        attn_self(qT1, kT1, v1sb, aT1, 1, [3])
        pa1c(0)
        attn_cross(lazy["qT2"], 0, [0, 1])
        pa1a(1)
        attn_cross(lazy["qT2"], 0, [2, 3])
        pa1b(1)
        pa2a1(0)
        pa2a2(0)
        pa1c(1)
        # gelu c0 runs as one act-table-10 block; cross-attn c1 (table 6)
        # then overlaps the FF-c0 DVE/PE tail (ft8, wf matmuls, store)
        pa2b(0, [0, 1, 2, 3])
        attn_cross(lazy["qT2"], 1, [0, 1])
        pa2b(0, [4, 5, 6, 7])
        attn_cross(lazy["qT2"], 1, [2, 3])
        pa2a1(1)
        ff_out(0)
        store(0, AF.Identity)
        pa2a2(1)
        pa2b(1, [0, 1, 2, 3, 4, 5, 6, 7])
        ff_out(1)
        store(1, AF.Copy)
        if dbg:
            dump("d_qT1", qT1[:])
            dump("d_kT1", kT1[:])
            dump("d_v1", v1sb[:])
            dump("d_aT1", aT1[:])
            dump("d_xT1", lazy["xT1"][:])
            dump("d_qT2", lazy["qT2"][:])
            dump("d_aT2", lazy["aT2"][:])
            dump("d_xT2", lazy["xT2"][:])
            dump("d_z3", lazy["z3"][:])
            dump("d_fft0", fft2[0][0][:])
            dump("d_kT2", kT2[:])
            dump("d_v2", v2sb[:])
            dump("d_encT", encT[:])

    return nc


_CACHED = {}


def _prep(inputs):
    """Host-side: fold LN gains, scale+convert weights to fp8, compute flags."""
    f32 = {k: np.ascontiguousarray(np.asarray(v), dtype=np.float32)
           for k, v in inputs.items()}
    g1, g2, g3 = f32["ln1_g"], f32["ln2_g"], f32["ln3_g"]
    b1, b2, b3 = f32["ln1_b"], f32["ln2_b"], f32["ln3_b"]
    wf32 = {
        "wq1": f32["wq1"] * g1[:, None], "wk1": f32["wk1"] * g1[:, None],
        "wv1": f32["wv1"] * g1[:, None], "wo1": f32["wo1"],
        "wq2": f32["wq2"] * g2[:, None], "wk2": f32["wk2"],
        "wv2": f32["wv2"], "wo2": f32["wo2"],
        "wg": f32["wg"] * g3[:, None], "wf": f32["wf"],
    }
    ws = {}
    w8 = {}
    for n, w in wf32.items():
        amax = float(np.abs(w).max()) or 1.0
        s = int(math.floor(math.log2(200.0 / amax)))
        ws[n] = s
        w8[n] = np.clip(w * (2.0 ** s), -240.0, 240.0).astype(
            ml_dtypes.float8_e4m3)
    bias = {
        "bq1": b1 @ f32["wq1"], "bk1": b1 @ f32["wk1"], "bv1": b1 @ f32["wv1"],
        "bq2": b2 @ f32["wq2"],
        "bo1": f32["bo1"], "bo2": f32["bo2"], "bf": f32["bf"],
        "bgu": b3 @ f32["wg"][:, :FF] + f32["bg"][:FF],
        "bgg": b3 @ f32["wg"][:, FF:] + f32["bg"][FF:],
    }
    flags = {n: bool(np.any(v != 0.0)) for n, v in bias.items()}
    dev_bias = {
        "bq1": bias["bq1"] * (SX * 2.0 ** ws["wq1"]),
        "bk1": bias["bk1"] * (SX * 2.0 ** ws["wk1"]),
        "bv1": bias["bv1"] * (SX * 2.0 ** ws["wv1"]),
        "bq2": bias["bq2"] * (SX * 2.0 ** ws["wq2"]),
        "bo1": bias["bo1"] * (SA * 2.0 ** ws["wo1"]),
        "bo2": bias["bo2"] * (SA * 2.0 ** ws["wo2"]),
        "bf": bias["bf"] * (SX * 2.0 ** ws["wf"]),
        "bgu": bias["bgu"] * (SX * 2.0 ** ws["wg"]),
        "bgg": bias["bgg"],    # true units (gelu bias)
    }
    return f32, w8, ws, dev_bias, flags


def _get_nc(key=None, ws=None, flags=None):
    if key is None:
        assert _CACHED, "kernel() must run before timeline queries"
        return next(iter(_CACHED.values()))
    if key not in _CACHED:
        nc = bacc.Bacc("TRN2", target_bir_lowering=False, debug=False,
                       num_devices=B)
        # bacc's act-table auto-pass maps Ln->natural_log and
        # Exp->exp_and_others (first table containing each func) and inserts
        # 15+ alternating table loads (~1.3us ACT each) across the
        # LN/softmax interleave. Feed it a doctored table list where only
        # natural_log_exp_and_others (Ln+Exp, one table for all of
        # layernorm+softmax) and gelu_and_others (Gelu+Copy, FF tail +
        # ACT store-copies) are matchable; emitted act_func_set_ids keep
        # their canonical positions so walrus' runtime remap is unchanged.
        import bass_rust as _bass_rust
        from concourse.hw_specs import get_activation_tables as _gat

        def _patched_insert_act_loads(_nc=nc):
            tabs = list(_gat(_nc.m.arch).items())
            doctored = []
            for name, funcs in tabs:
                if name == "natural_log_exp_and_others":
                    doctored.append((name, set(funcs) - {AF.Copy}))
                elif name == "gelu_and_others":
                    doctored.append((name, set(funcs)))
                else:
                    doctored.append((name, set()))
            _bass_rust.insert_act_table_loads(_nc, doctored)

        nc.insert_act_table_loads = _patched_insert_act_loads
        build(nc, ws, flags)
        nc.finalize()
        _CACHED[key] = nc
    return _CACHED[key]


def kernel(**inputs) -> np.ndarray:
    f32, w8, ws, dev_bias, flags = _prep(inputs)
    key = (tuple(sorted(flags.items())), tuple(sorted(ws.items())))
    nc = _get_nc(key, ws, flags)
    shared = dict(w8)
    for n, v in dev_bias.items():
        shared[n] = np.ascontiguousarray(v, dtype=np.float32)
    in_maps = [dict(shared, x=np.ascontiguousarray(f32["x"][i]),
                    enc=np.ascontiguousarray(f32["enc"][i])) for i in range(B)]
    res = run_bass_kernel_spmd(nc, in_maps, core_ids=list(range(B)))
    return np.stack([res.results[i]["out"] for i in range(B)], axis=0)


if __name__ == "__main__":
    print("module import ok")



# revision 64
# speedup vs baseline: 1.0392x; 1.0392x over previous
"""Trainium2 Bass kernel for a BasicTransformerBlock (self-attn + cross-attn + GEGLU FF).

Sharding: data-parallel over the batch axis - 8 batch elements onto 8 NeuronCores,
same SPMD program, no collectives.

v3 design (v2 + act-table + scheduling/balance work; 247us -> 233us):
- Heavy matmuls run fp8e4 (TRN e4m3) with DoubleRow perf mode (0.5 cyc/row, K=256
  per instruction). Weights are power-of-2-scaled + converted to fp8 on the HOST
  and DMA'd straight into SBUF; QKV1 weights DMA first so LN1->projections are
  never input-starved.
- Residual stream stays transposed f32r xT[d, s]. LayerNorm gain g folds into
  the weights host-side. 1/std folds ONCE into zv1 = (x-mu)*SX*rstd (column
  scaling commutes through feature-mixing matmuls), so q/k/v all project from
  zv1 and their PSUM->SBUF evacuations are pure scales on the (then-idle) ACT
  engine as Identity ops. x psum->sbuf restage also runs on ACT at load time.
- Softmax: scores land in 2-bank PSUM pairs, one ACT exp per pair writing fp8
  probs 16*e^s in DoubleRow [p,2,n] layout; denominators via ones-lhsT
  DoubleRow matmuls. aT stays head-dim-major [64, NH, S] (walrus rejects any
  nonzero matmul psum dst partition); normalization = one DVE reciprocal +
  Pool partition-broadcasts + one fused stt per (c, hf).
- Activation tables: bacc's auto-pass is fed a doctored table list where only
  natural_log_exp_and_others (Ln/Exp/Identity) and gelu_and_others (Gelu/Copy)
  are matchable, collapsing 19 alternating table loads (~24us ACT) to 2-3.
  Gelus run as contiguous per-chunk blocks; ACT copies use Identity in the
  Ln/Exp phase and Copy in the gelu phase.
- Scales: activations x16, probs x16, attention out x64, per-weight 2^s with
  s = floor(log2(200/absmax)).
- Software pipelining: attention tails deferred one hf; encoder K/V block
  emitted inside the self-attention window; GEGLU chunk 0 emitted before
  cross-attention chunk 1 so the FF-c0 DVE/PE tail overlaps the thin
  cross-attn latency chains; FF psum g-tiles alternate between the psP slot
  and the stats slot (double-buffering); LN xsq and zv multiplies split
  across DVE and Pool. The LN mu-broadcast psum and the mo==0 / even-tp
  projection psums rotate through the 1-buf stats slot instead of the score
  slots, so attention score matmuls (which feed the ACT-critical exp stream)
  are never starved of psum during the LN2/LN3/v-proj windows.
"""
import sys

sys.path.insert(0, "/opt/trn_rl_repo")

import math
from contextlib import ExitStack

import ml_dtypes
import numpy as np

import concourse.bass as bass
import concourse.mybir as mybir
import concourse.tile as tile
from concourse import bacc
from concourse.bass_utils import run_bass_kernel_spmd
from concourse.masks import make_identity

F32 = mybir.dt.float32
F32R = mybir.dt.float32r
FP8 = mybir.dt.float8e4
AF = mybir.ActivationFunctionType
ALU = mybir.AluOpType
PM = mybir.MatmulPerfMode

B = 8
S = 1024          # tokens
D = 512           # model dim
SK2 = 77          # cross-attention source length
DE = 768          # encoder dim
FF = 2048         # GEGLU inner dim (per half)
NH = 8            # heads
DH = 64           # head dim
DHP = 80          # padded head slot in v tiles (16B-aligned pair strides);
                  # col 64 holds the ones column that folds the softmax
                  # denominator into the pv matmul (M=65, den = pv[64])
SCALE = DH ** -0.5
EPS = 1e-5
P = 128
NC = 512          # token chunk (one psum bank of fp32)
ST = S // P       # 8 token tiles
FT = D // P       # 4 feature tiles
CH = S // NC      # 2 token chunks
KE = DE // P      # 6 encoder feature tiles
NI = FF // P      # 16 FF inner tiles

SX = 16.0         # activation scale (z, enc, q, k, v)
SEXP = 16.0       # probs scale
SA = 64.0         # attention-output scale
LN_SEXP = math.log(SEXP)

WNAMES = ["wq1", "wk1", "wv1", "wo1", "wq2", "wk2", "wv2", "wo2", "wg", "wf"]


def build(nc: bass.Bass, ws: dict, flags: dict, dbg=False):
    """ws: name -> log2 weight scale. flags: name -> bias vector is nonzero."""
    x = nc.dram_tensor("x", [S, D], F32, kind="ExternalInput")
    enc = nc.dram_tensor("enc", [SK2, DE], F32, kind="ExternalInput")
    w_in = {}
    for name, shape in [
        ("wq1", [D, D]), ("wk1", [D, D]), ("wv1", [D, D]), ("wo1", [D, D]),
        ("wq2", [D, D]), ("wk2", [DE, D]), ("wv2", [DE, D]), ("wo2", [D, D]),
        ("wg", [D, 2 * FF]), ("wf", [FF, D]),
    ]:
        w_in[name] = nc.dram_tensor(name, shape, FP8, kind="ExternalInput")
    vec_in = {}
    for name, n in [("bq1", D), ("bk1", D), ("bv1", D), ("bq2", D),
                    ("bo1", D), ("bo2", D), ("bf", D),
                    ("bgu", FF), ("bgg", FF)]:
        vec_in[name] = nc.dram_tensor(name, [n], F32, kind="ExternalInput")
    out = nc.dram_tensor("out", [S, D], F32, kind="ExternalOutput")
    dbg_out = {}
    if dbg:
        for name, shape, dt in [
            ("d_qT1", [P, FT, S], FP8), ("d_kT1", [P, FT, S], FP8),
            ("d_v1", [P, ST // 2, 2, NH, DH], FP8),
            ("d_aT1", [DH, NH, S], FP8), ("d_xT1", [P, FT, S], F32R),
            ("d_qT2", [P, FT, S], FP8), ("d_aT2", [DH, NH, S], FP8),
            ("d_xT2", [P, FT, S], F32R), ("d_z3", [P, FT, S], FP8),
            ("d_fft0", [P, 2, NC], FP8), ("d_kT2", [P, FT, 80], FP8),
            ("d_rc", [1, 2, NC], F32), ("d_bc", [DH, 2, NC], F32),
            ("d_v2", [SK2, NH, DH], FP8), ("d_encT", [P, KE, 80], FP8),
        ]:
            dbg_out[name] = nc.dram_tensor(name, shape, dt,
                                           kind="ExternalOutput")

    cw = {n: 2.0 ** -ws[n] for n in WNAMES}   # descale constants

    with tile.TileContext(nc) as tc, ExitStack() as es:
        const = es.enter_context(tc.tile_pool(name="const", bufs=1))
        resid = es.enter_context(tc.tile_pool(name="resid", bufs=2))
        zp = es.enter_context(tc.tile_pool(name="zp", bufs=2))
        att = es.enter_context(tc.tile_pool(name="att", bufs=1))
        wp = es.enter_context(tc.tile_pool(name="wp", bufs=1))
        stage = es.enter_context(tc.tile_pool(name="stage", bufs=2))
        t3p = es.enter_context(tc.tile_pool(name="t3p", bufs=1))
        rowp = es.enter_context(tc.tile_pool(name="rowp", bufs=1))
        bcp = es.enter_context(tc.tile_pool(name="bcp", bufs=3))
        exp_p = es.enter_context(tc.tile_pool(name="exp", bufs=3))
        gelp = es.enter_context(tc.tile_pool(name="gelp", bufs=2))
        fftp = es.enter_context(tc.tile_pool(name="fftp", bufs=8))
        psA = es.enter_context(tc.tile_pool(name="psA", bufs=2, space="PSUM"))
        psP = es.enter_context(tc.tile_pool(name="psP", bufs=1, space="PSUM"))

        # ---- constants ----
        ident = const.tile([P, P], F32, tag="ident")
        make_identity(nc, ident[:])
        ones_f = const.tile([P, P], F32, tag="ones_f")
        nc.vector.memset(ones_f[:], 1.0)
        ones128 = const.tile([P, 1], F32R, tag="o128")  # stats lhsT (K=128,M=1)
        nc.vector.tensor_copy(ones128[:], ones_f[:, 0:1])
        ones1x = const.tile([1, P], F32R, tag="o1x")    # bcast lhsT (K=1)
        nc.vector.tensor_copy(ones1x[:], ones_f[0:1, :])
        ones_row = None
        if any(flags.values()):
            ones_row_f = const.tile([1, NC], F32, tag="orowf")
            nc.vector.memset(ones_row_f[:], 1.0)
            ones_row = const.tile([1, NC], F32R, tag="orow")  # rank-1 bias rhs
            nc.vector.tensor_copy(ones_row[:], ones_row_f[:])
        ones8 = const.tile([P, 2, 16], FP8, tag="o8")   # self denominator lhsT
        nc.vector.memset(ones8[:], 1.0)                 # (16B-aligned pair stride)
        ones2 = const.tile([SK2, 1], FP8, tag="o2")     # cross denominator lhsT
        nc.vector.memset(ones2[:], 1.0)
        lnsc = const.tile([P, 1], F32, tag="lnsc")      # exp bias = ln(SEXP)
        nc.vector.memset(lnsc[:], LN_SEXP)
        eps_t = const.tile([1, 1], F32, tag="eps")
        nc.vector.memset(eps_t[:], EPS)
        # explicit activation-table management: one Ln+Exp table for all of
        # layernorm + softmax, a single switch to the gelu table for the FF
        # tail (Copy is in both tables, so ACT evacuations never force a
        # switch). bacc's auto-insertion pass is disabled in _get_nc — it
        # maps Ln->natural_log and Exp->exp_and_others and thrashes 15+
        # table loads (~1.3us ACT each) across the LN/softmax interleave.
        try:
            from concourse.hw_specs import get_activation_tables
            _t = list(get_activation_tables(nc.m.arch))
            TAB_LNEXP = _t.index("natural_log_exp_and_others")
            TAB_GELU = _t.index("gelu_and_others")
        except Exception:
            TAB_LNEXP, TAB_GELU = 6, 10

        def load_table(tid):
            if tid is not None:
                nc.scalar.add_instruction(mybir.InstLoadActFuncSet(
                    name=nc.get_next_instruction_name(), ins=[], outs=[],
                    act_func_set_id=tid))

        warm = const.tile([1, 1], F32, tag="warm")
        nc.scalar.activation(warm[:], eps_t[:], AF.Ln)   # pull act-table load
        nc.scalar.activation(warm[:], warm[:], AF.Exp)   # into the DMA window

        # ---- weights: DMA fp8 straight into SBUF ----
        def load_w(name, kouter):
            dram = w_in[name]
            wr = wp.tile([P, kouter, dram.shape[1]], FP8, tag=f"{name}r")
            nc.sync.dma_start(wr[:], dram.rearrange("(ko ki) n -> ki ko n", ki=P))
            return wr

        bgg_c = const.tile([P, NI], F32, tag="bgg_c")
        if flags["bgg"]:
            nc.sync.dma_start(bgg_c[:],
                              vec_in["bgg"].rearrange("(o p) -> p o", p=P))

        def bias_row(name):
            """[1, N] f32r row of host-scaled bias, for rank-1 lhsT slices."""
            n = vec_in[name].shape[0]
            tf = const.tile([1, n], F32, tag=f"{name}_f")
            nc.sync.dma_start(tf[:], vec_in[name].rearrange("(o n) -> o n", o=1))
            tr = const.tile([1, n], F32R, tag=f"{name}_r")
            nc.vector.tensor_copy(tr[:], tf[:])
            return tr

        brow = {name: bias_row(name)
                for name in ("bq1", "bk1", "bv1", "bq2", "bo1", "bo2", "bf",
                             "bgu") if flags[name]}

        # ---- load x, transpose into xT [128, FT, S] (f32r) ----
        # x and enc DMAs go first so the transposes are never input-starved;
        # weight DMAs queue behind them and overlap LN1/QKV1 compute.
        xT = resid.tile([P, FT, S], F32R, tag="x")
        x_stage = []
        for st in range(ST):
            xr = stage.tile([P, D], F32, tag="x_raw", bufs=3)
            nc.sync.dma_start(xr[:], x[P * st:P * (st + 1), :])
            x_stage.append(xr)
        def load_wo(name):
            dram = w_in[name]
            wr = wp.tile([DH, NH, dram.shape[1]], FP8, tag=f"{name}r")
            nc.sync.dma_start(wr[:], dram.rearrange("(h k) n -> k h n", k=DH))
            return wr

        wq1r = load_w("wq1", FT)
        wk1r = load_w("wk1", FT)
        wv1r = load_w("wv1", FT)
        enc_raw = const.tile([SK2, DE], F32, tag="enc_raw")
        nc.sync.dma_start(enc_raw[:], enc[:, :])
        wo1r = load_wo("wo1")
        wq2r = load_w("wq2", FT)
        wk2r = load_w("wk2", KE)
        wv2r = load_w("wv2", KE)
        wo2r = load_wo("wo2")
        wgr = load_w("wg", FT)
        wfr = load_w("wf", NI)
        def load_x_pair(tp):
            pt = psA.tile([P, 2, NC], F32, tag="pA")
            for half in range(2):
                st = 2 * tp + half
                xr = x_stage[st]
                for ft in range(FT):
                    nc.tensor.transpose(pt[:, half, P * ft:P * (ft + 1)],
                                        xr[:, P * ft:P * (ft + 1)], ident[:])
            # psum->xT restage on ACT (idle during the load phase;
            # Identity is in the Ln/Exp table so no act-table switch)
            nc.scalar.activation(
                xT[:, :, 2 * P * tp:2 * P * (tp + 1)].rearrange(
                    "p f (a q) -> p a f q", a=2),
                pt[:].rearrange("p a (f q) -> p a f q", f=FT), AF.Identity)

        # ---- enc -> encT [128, KE, 80] fp8 (x16), cols 77:80 zero-padded
        # (pair slices need 16B-aligned outer stride for dual-fp8 ldweights).
        # Emitted later, during ACT-bound self-attention: only cross-attn
        # consumes these, and their DVE copies would otherwise delay LN1. ----
        SK2P = 80
        SK2A = 80   # padded so [hp, hf, :] slices stay 4B-aligned
        encT = att.tile([P, KE, SK2P], FP8, tag="encT")
        kT2 = att.tile([P, FT, SK2A], FP8, tag="kT2")
        v2sb = att.tile([SK2, NH, DH], FP8, tag="v2sb")

        def emit_enc_block():
            pass

        def _enc_block():
            nc.vector.memset(encT[:, :, SK2:SK2P], 0.0)
            for kp in range(KE // 2):
                pt = psA.tile([P, 2, NC], F32, tag="pA")
                for half in range(2):
                    ke = 2 * kp + half
                    nc.tensor.transpose(pt[:, half, 0:SK2],
                                        enc_raw[:, P * ke:P * (ke + 1)],
                                        ident[0:SK2, 0:SK2])
                nc.vector.tensor_scalar_mul(
                    encT[:, 2 * kp:2 * kp + 2, 0:SK2], pt[:, :, 0:SK2], SX)
            for mp in range(FT // 2):
                pt = psA.tile([P, 2, NC], F32, tag="pA")
                for half in range(2):
                    mo = 2 * mp + half
                    for t in range(KE // 2):
                        nc.tensor.matmul(pt[:, half, 0:SK2P],
                                         wk2r[:, 2 * t:2 * t + 2,
                                              P * mo:P * (mo + 1)],
                                         encT[:, 2 * t:2 * t + 2, :],
                                         start=(t == 0),
                                         stop=(t == KE // 2 - 1),
                                         perf_mode=PM.DoubleRow)
                nc.vector.tensor_scalar_mul(kT2[:, 2 * mp:2 * mp + 2, 0:SK2],
                                            pt[:, :, 0:SK2], cw["wk2"])
            ptv = psA.tile([P, 2, NC], F32, tag="pA", name="ptv2")
            for t in range(KE // 2):
                nc.tensor.matmul(ptv[0:SK2P, 0, :],
                                 encT[:, 2 * t:2 * t + 2, :],
                                 wv2r[:, 2 * t:2 * t + 2, :],
                                 start=(t == 0), stop=(t == KE // 2 - 1),
                                 perf_mode=PM.DoubleRow)
            nc.vector.tensor_scalar_mul(
                v2sb[:].rearrange("s h d -> s (h d)"), ptv[0:SK2, 0, :],
                cw["wv2"])

        # ---- LayerNorm: stats + rows; z via gpsimd (all-SBUF) ----
        def ln_sums(src, c, tagpfx):
            cs = slice(NC * c, NC * (c + 1))
            sum_ps = psA.tile([1, 2, NC], F32, tag="den", bufs=1,
                              name=f"st_{tagpfx}{c}")
            for ft in range(FT):
                nc.tensor.matmul(sum_ps[:, 0, :], ones128[:], src[:, ft, cs],
                                 start=(ft == 0), stop=(ft == FT - 1))
            for ft in range(FT):
                xsq = stage.tile([P, NC], F32R, tag="xsq", bufs=3)
                sq_eng = nc.vector if (tagpfx == "l1" or ft < 2) else nc.gpsimd
                sq_eng.tensor_tensor(xsq[:], src[:, ft, cs], src[:, ft, cs],
                                     ALU.mult)
                nc.tensor.matmul(sum_ps[:, 1, :], ones128[:], xsq[:],
                                 start=(ft == 0), stop=(ft == FT - 1),
                                 skip_group_check=True)
            return sum_ps

        def ln_stats(src, c, tagpfx, rstd_dtype=F32, sum_ps=None):
            """-> (mu16_sb [P,NC], rstd row, lnv row) for chunk c of src."""
            if sum_ps is None:
                sum_ps = ln_sums(src, c, tagpfx)
            mu = rowp.tile([1, NC], F32, tag="mu", bufs=2)
            nc.vector.tensor_scalar_mul(mu[:], sum_ps[:, 0, :], 1.0 / D)
            mu16 = rowp.tile([1, NC], F32R, tag="mu16")
            nc.vector.tensor_scalar_mul(mu16[:], mu[:], SX)
            musq = rowp.tile([1, NC], F32, tag="musq")
            nc.gpsimd.tensor_tensor(musq[:], mu[:], mu[:], ALU.mult)
            var = rowp.tile([1, NC], F32, tag="var", bufs=2)
            nc.vector.scalar_tensor_tensor(var[:], sum_ps[:, 1, :], 1.0 / D,
                                           musq[:], op0=ALU.mult,
                                           op1=ALU.subtract)
            lnv = rowp.tile([1, NC], F32, tag="lnv", bufs=2)
            nc.scalar.activation(lnv[:], var[:], AF.Ln, bias=eps_t[:])
            rstd = rowp.tile([1, NC], rstd_dtype, tag="rstd", bufs=2)
            nc.scalar.activation(rstd[:], lnv[:], AF.Exp, scale=-0.5)
            # rotate through the stats slot rather than a score slot so
            # attention score matmuls aren't starved during LN2/LN3
            mbt = psA.tile([P, 2, NC], F32, tag="den", bufs=1,
                           name=f"mub_{tagpfx}{c}")
            nc.tensor.matmul(mbt[:, 0, :], ones1x[:], mu16[:], start=True,
                             stop=True)
            return mbt[:, 0:1, :], rstd, lnv

        def z_from(src, mu16_b, c, zt):
            cs = slice(NC * c, NC * (c + 1))
            for mo in range(0, FT, 2):
                nc.vector.scalar_tensor_tensor(
                    zt[:, mo:mo + 2, cs], src[:, mo:mo + 2, cs], SX,
                    mu16_b.broadcast_to([P, 2, NC]),
                    op0=ALU.mult, op1=ALU.subtract)

        def sd_row_from(lnv):
            sd = rowp.tile([1, NC], F32R, tag="sd", bufs=2)
            nc.scalar.activation(sd[:], lnv[:], AF.Exp, scale=0.5)
            return sd

        # ---- projection helpers ----
        def project_pair(w_r, wname, zt, c, mo, dst, rstd_sb=None, bname=None,
                         sd_row=None, evac="act"):
            """dst[:, mo:mo+2, cs] = (W.T@z * cw) [* rstd] (+ bias via rank-1).
            evac: 'act' = ACT Identity (pure scale; zt must carry rstd
            already), 'dve' = DVE tensor_scalar (pure scale), 'dve_rstd' =
            DVE stt with per-token rstd (zt un-rstd'd)."""
            cs = slice(NC * c, NC * (c + 1))
            pp = psA.tile([P, 2, NC], F32, tag="den" if mo == 0 else "pA",
                          bufs=1 if mo == 0 else 2)
            for half in range(2):
                m = mo + half
                for t in range(FT // 2):
                    nc.tensor.matmul(pp[:, half, :],
                                     w_r[:, 2 * t:2 * t + 2, P * m:P * (m + 1)],
                                     zt[:, 2 * t:2 * t + 2, cs],
                                     start=(t == 0), stop=(t == FT // 2 - 1),
                                     perf_mode=PM.DoubleRow)
                if bname is not None and bname in brow:
                    nc.tensor.matmul(pp[:, half, :],
                                     brow[bname][:, P * m:P * (m + 1)],
                                     sd_row[:], start=False, stop=True,
                                     skip_group_check=True)
            if evac == "act":
                nc.scalar.activation(dst[:, mo:mo + 2, cs], pp[:],
                                     AF.Identity, scale=cw[wname])
            elif evac == "dve":
                nc.vector.tensor_scalar_mul(dst[:, mo:mo + 2, cs], pp[:],
                                            cw[wname])
            else:
                nc.vector.scalar_tensor_tensor(
                    dst[:, mo:mo + 2, cs], pp[:], cw[wname],
                    rstd_sb[:, None, :].broadcast_to([P, 2, NC]),
                    op0=ALU.mult, op1=ALU.mult)

        def out_proj_pair(w_r, wname, aT, c, mo, src, dst, post, bname=None):
            """dst[:, mo:mo+2, cs] = src + (W.T@a * cw * post) [+ bias].
            aT is head-dim-major [DH, NH, S]; contraction runs over head
            pairs (K = 2*DH per DoubleRow matmul)."""
            cs = slice(NC * c, NC * (c + 1))
            pp = psA.tile([P, 2, NC], F32, tag="den" if mo == 0 else "pA",
                          bufs=1 if mo == 0 else 2)
            for half in range(2):
                m = mo + half
                for t in range(NH // 2):
                    nc.tensor.matmul(pp[:, half, :],
                                     w_r[:, 2 * t:2 * t + 2, P * m:P * (m + 1)],
                                     aT[:, 2 * t:2 * t + 2, cs],
                                     start=(t == 0), stop=(t == NH // 2 - 1),
                                     perf_mode=PM.DoubleRow)
                if bname is not None and bname in brow:
                    nc.tensor.matmul(pp[:, half, :],
                                     brow[bname][:, P * m:P * (m + 1)],
                                     ones_row[:], start=False, stop=True,
                                     skip_group_check=True)
            nc.vector.scalar_tensor_tensor(
                dst[:, mo:mo + 2, cs], pp[:], cw[wname] * post,
                src[:, mo:mo + 2, cs], op0=ALU.mult, op1=ALU.add)

        # ---- attention (self) for one chunk ----
        # rc_act: cross-attn tails land in the (DVE-bound, ACT-idle) FF
        # window - compute 1/den as Exp(-Ln(den)) on ACT there instead of
        # a DVE reciprocal. Self-attn tails run while ACT is exp-saturated,
        # so they keep the DVE reciprocal.
        def attn_tail(aT, c, hf, pv, den):
            cs = slice(NC * c, NC * (c + 1))
            rc = rowp.tile([1, 2, NC], F32, tag="rc", bufs=2)
            nc.vector.reciprocal(rc[:], den[:])
            bc = bcp.tile([DH, 2, NC], F32, tag="bc", bufs=3)
            nc.gpsimd.partition_broadcast(bc[:, 0, :], rc[:, 0, :])
            nc.gpsimd.partition_broadcast(bc[:, 1, :], rc[:, 1, :])
            nc.vector.scalar_tensor_tensor(
                aT[:, 2 * hf:2 * hf + 2, cs], pv[:], SA / SEXP,
                bc[:], op0=ALU.mult, op1=ALU.mult)

        pend = []   # software-pipelined attention tails across calls

        def flush_tail():
            if pend:
                attn_tail(*pend.pop())

        def attn_self(qT, kT, v1sb, aT, c, hfs, mid=None):
            cs = slice(NC * c, NC * (c + 1))
            for hf in hfs:
                exs = []
                for par in range(2):
                    hp = slice(DH * par, DH * par + DH)
                    for j in range(ST // 2):
                        if mid is not None and hf == hfs[0]:
                            if par == 0 and j == 2:
                                mid(0)
                            elif par == 1 and j == 0:
                                mid(1)
                        sc = psA.tile([P, 2, NC], F32, tag="pA")
                        for half in range(2):
                            sk = 2 * j + half
                            nc.tensor.matmul(
                                sc[:, half, :],
                                kT[hp, hf, P * sk:P * (sk + 1)],
                                qT[hp, hf, cs], start=True, stop=True)
                        ex = exp_p.tile([P, 2, NC], FP8, tag="ex", bufs=6)
                        nc.scalar.activation(ex[:], sc[:], AF.Exp,
                                             scale=SCALE / (SX * SX),
                                             bias=lnsc[:])
                        exs.append(ex)
                flush_tail()
                pv = psP.tile([DH, 2, NC], F32, tag="pvp",
                              name=f"pv1_{c}_{hf}")
                den = psA.tile([1, 2, NC], F32, tag="den", bufs=1,
                               name=f"den1_{c}_{hf}")
                for par in range(2):
                    h = 2 * hf + par
                    for j in range(ST // 2):
                        ex = exs[par * (ST // 2) + j]
                        nc.tensor.matmul(pv[:, par, :],
                                         v1sb[:, j, :, h, :], ex[:],
                                         start=(j == 0),
                                         stop=(j == ST // 2 - 1),
                                         perf_mode=PM.DoubleRow,
                                         skip_group_check=True)
                        nc.tensor.matmul(den[:, par, :], ones8[:, :, 0:1], ex[:],
                                         start=(j == 0), stop=(j == ST // 2 - 1),
                                         perf_mode=PM.DoubleRow,
                                         skip_group_check=True)
                pend.append((aT, c, hf, pv, den))

        # ---- attention (cross) for one chunk ----
        def attn_cross(qT, c, hfs):
            aT = lz("aT2", lambda: att.tile([DH, NH, S], FP8, tag="ka",
                                            bufs=1, name="aT2"))
            cs = slice(NC * c, NC * (c + 1))
            for hf in hfs:
                sc = psA.tile([P, 2, NC], F32, tag="pA")
                for par in range(2):
                    hp = slice(DH * par, DH * par + DH)
                    nc.tensor.matmul(sc[0:SK2, par, :], kT2[hp, hf, 0:SK2],
                                     qT[hp, hf, cs], start=True, stop=True)
                ex = exp_p.tile([SK2, 2, NC], FP8, tag="ex2c", bufs=2)
                nc.scalar.activation(ex[:], sc[0:SK2, :, :], AF.Exp,
                                     scale=SCALE / (SX * SX),
                                     bias=lnsc[0:SK2, :])
                flush_tail()
                pv = psP.tile([DH, 2, NC], F32, tag="pvp",
                              name=f"pv2_{c}_{hf}")
                den = psA.tile([1, 2, NC], F32, tag="den", bufs=1,
                               name=f"den2_{c}_{hf}")
                for par in range(2):
                    h = 2 * hf + par
                    nc.tensor.matmul(pv[:, par, :],
                                     v2sb[:, h, :], ex[:, par, :],
                                     start=True, stop=True,
                                     skip_group_check=True)
                    nc.tensor.matmul(den[:, par, :], ones2[:], ex[:, par, :],
                                     start=True, stop=True,
                                     skip_group_check=True)
                pend.append((aT, c, hf, pv, den))

        # ================= LN1 + QKV (both chunks) =================
        # rstd folds into zv1 once (column scaling commutes through the
        # feature-mixing projections), so q/k/v all project from zv1 and
        # their evacuations become pure scales (ACT Identity / DVE scale)
        zv1 = zp.tile([P, FT, S], FP8, tag="z")
        qT1 = att.tile([P, FT, S], FP8, tag="qt", bufs=1)
        kT1 = att.tile([P, FT, S], FP8, tag="ka", bufs=1)
        v1sb = att.tile([P, ST // 2, 2, NH, DH], FP8, tag="v1sb")
        rstd1_sb = {}
        sd1 = {}
        need_sd1 = flags["bq1"] or flags["bk1"] or flags["bv1"]
        def v_proj_pair(c, tp):
            pp = psA.tile([P, 2, NC], F32, tag="den" if tp % 2 == 0 else "pA",
                          bufs=1 if tp % 2 == 0 else 2)
            for half in range(2):
                stt = 2 * tp + half
                for t in range(FT // 2):
                    nc.tensor.matmul(
                        pp[:, half, :],
                        zv1[:, 2 * t:2 * t + 2, P * stt:P * (stt + 1)],
                        wv1r[:, 2 * t:2 * t + 2, :],
                        start=(t == 0), stop=(t == FT // 2 - 1),
                        perf_mode=PM.DoubleRow)
                if flags["bv1"]:
                    off = P * stt - NC * c
                    nc.tensor.matmul(
                        pp[:, half, :],
                        sd1[c][:, off:off + P], brow["bv1"][:],
                        start=False, stop=True, skip_group_check=True)
            nc.vector.tensor_scalar_mul(
                v1sb[:, tp, :, :, :].rearrange("p a h d -> p (a h d)"),
                pp[:].rearrange("p a n -> p (a n)"), cw["wv1"])

        for tp in range(ST // 2):
            load_x_pair(tp)
        mu16b1 = {}
        for c in range(CH):
            cs = slice(NC * c, NC * (c + 1))
            mu16_b, rstd, lnv = ln_stats(xT, c, "l1")
            mu16b1[c] = mu16_b
            rsb = bcp.tile([P, NC], F32, tag="rstdb", name=f"rstd1b_{c}")
            nc.gpsimd.partition_broadcast(rsb[:], rstd[:])
            rstd1_sb[c] = rsb
            if need_sd1:
                sd1[c] = sd_row_from(lnv)
            for mo in range(0, FT, 2):
                t1 = t3p.tile([P, 2, NC], F32, tag="t1", bufs=2)
                nc.vector.scalar_tensor_tensor(
                    t1[:], xT[:, mo:mo + 2, cs], SX,
                    mu16_b.broadcast_to([P, 2, NC]),
                    op0=ALU.mult, op1=ALU.subtract)
                zv_eng = nc.vector if mo == 0 else nc.gpsimd
                zv_eng.tensor_tensor(
                    zv1[:, mo:mo + 2, cs], t1[:],
                    rsb[:, None, :].broadcast_to([P, 2, NC]), ALU.mult)
        # K projections first (attention needs all keys), then q(c0)/v, then
        # q(c1) is deferred into the first attention call
        for c in range(CH):
            for mo in range(0, FT, 2):
                project_pair(wk1r, "wk1", zv1, c, mo, kT1, None,
                             "bk1", sd1.get(c), evac="act")
        for mo in range(0, FT, 2):
            project_pair(wq1r, "wq1", zv1, 0, mo, qT1, None,
                         "bq1", sd1.get(0), evac="act")

        def emit_v_projs(half):
            for tp in (2 * half, 2 * half + 1):
                v_proj_pair(tp // 2, tp)

        # ================= chunk-pipelined main pass =================
        # tiles are allocated lazily at first use so the resid/z/ka tag
        # rotations only reclaim slots whose previous tenant is dead
        aT1 = att.tile([DH, NH, S], FP8, tag="aT1")
        lazy = {}

        def lz(name, ctor):
            if name not in lazy:
                lazy[name] = ctor()
            return lazy[name]

        rstd2_sb = {}
        sd2 = {}
        fft2 = {0: [], 1: []}

        def pa1a(c):
            flush_tail()
            xT1 = lz("xT1", lambda: resid.tile([P, FT, S], F32R, tag="x",
                                               name="xT1"))
            for mo in range(0, FT, 2):
                out_proj_pair(wo1r, "wo1", aT1, c, mo, xT, xT1, 1.0 / SA, "bo1")

        def pa1b(c):
            xT1 = lazy["xT1"]
            mu16_sb, rstd, lnv = ln_stats(xT1, c, "l2")
            rsb = bcp.tile([P, NC], F32, tag="rstdb", name=f"rstd2b_{c}")
            nc.gpsimd.partition_broadcast(rsb[:], rstd[:])
            rstd2_sb[c] = rsb
            if flags["bq2"]:
                sd2[c] = sd_row_from(lnv)
            z2 = lz("z2", lambda: zp.tile([P, FT, S], FP8, tag="z",
                                          name="z2"))
            z_from(xT1, mu16_sb, c, z2)

        def pa1c(c):
            z2 = lazy["z2"]
            qT2 = lz("qT2", lambda: att.tile([P, FT, S], FP8, tag="qt",
                                             bufs=1, name="qT2"))
            for mo in range(0, FT, 2):
                project_pair(wq2r, "wq2", z2, c, mo, qT2, rstd2_sb[c],
                             "bq2", sd2.get(c), evac="dve_rstd")

        def pa2a1(c):
            flush_tail()
            xT1 = lazy["xT1"]
            aT2 = lazy["aT2"]
            xT2 = lz("xT2", lambda: resid.tile([P, FT, S], F32R, tag="x",
                                               name="xT2"))
            for mo in range(0, FT, 2):
                out_proj_pair(wo2r, "wo2", aT2, c, mo, xT1, xT2, 1.0 / SA, "bo2")

        def pa2a2(c):
            cs = slice(NC * c, NC * (c + 1))
            xT2 = lazy["xT2"]
            z3 = lz("z3", lambda: zp.tile([P, FT, S], FP8, tag="z", name="z3"))
            mu16_sb, rstd, lnv = ln_stats(xT2, c, "l3")
            r3b = bcp.tile([P, NC], F32, tag="rstdb", name=f"rstd3b_{c}")
            nc.gpsimd.partition_broadcast(r3b[:], rstd[:])
            for mo in range(0, FT, 2):
                t3 = t3p.tile([P, 2, NC], F32, tag="t1", bufs=2)
                nc.vector.scalar_tensor_tensor(
                    t3[:], xT2[:, mo:mo + 2, cs], SX,
                    mu16_sb.broadcast_to([P, 2, NC]),
                    op0=ALU.mult, op1=ALU.subtract)
                nc.gpsimd.tensor_tensor(
                    z3[:, mo:mo + 2, cs], t3[:],
                    r3b[:, None, :].broadcast_to([P, 2, NC]), ALU.mult)

        def pa2b(c, js):
            # GEGLU inner: g/u pair tiles -> gelu -> fft (fp8 DoubleRow layout)
            cs = slice(NC * c, NC * (c + 1))
            z3 = lazy["z3"]
            for j in js:
                if j % 2 == 0:
                    pg = psP.tile([P, 2, NC], F32, tag="pvp",
                                  name=f"pg_{c}_{j}")
                else:
                    pg = psA.tile([P, 2, NC], F32, tag="den", bufs=1,
                                  name=f"pg_{c}_{j}")
                pu = psA.tile([P, 2, NC], F32, tag="pA")
                for half in range(2):
                    i = 2 * j + half
                    for t in range(FT // 2):
                        nc.tensor.matmul(pg[:, half, :],
                                         wgr[:, 2 * t:2 * t + 2,
                                             FF + P * i:FF + P * (i + 1)],
                                         z3[:, 2 * t:2 * t + 2, cs],
                                         start=(t == 0), stop=(t == FT // 2 - 1),
                                         perf_mode=PM.DoubleRow)
                for half in range(2):
                    i = 2 * j + half
                    for t in range(FT // 2):
                        nc.tensor.matmul(pu[:, half, :],
                                         wgr[:, 2 * t:2 * t + 2,
                                             P * i:P * (i + 1)],
                                         z3[:, 2 * t:2 * t + 2, cs],
                                         start=(t == 0), stop=(t == FT // 2 - 1),
                                         perf_mode=PM.DoubleRow)
                    if flags["bgu"]:
                        nc.tensor.matmul(pu[:, half, :],
                                         brow["bgu"][:, P * i:P * (i + 1)],
                                         ones_row[:], start=False, stop=True,
                                         skip_group_check=True)
                gel = gelp.tile([P, 2, NC], FP8, tag="gel")
                if flags["bgg"]:
                    for half in range(2):
                        i = 2 * j + half
                        nc.scalar.activation(gel[:, half, :], pg[:, half, :],
                                             AF.Gelu, scale=cw["wg"] / SX,
                                             bias=bgg_c[:, i:i + 1])
                else:
                    nc.scalar.activation(gel[:], pg[:], AF.Gelu,
                                         scale=cw["wg"] / SX)
                ft8 = fftp.tile([P, 2, NC], FP8, tag="ft8")
                nc.vector.scalar_tensor_tensor(ft8[:], pu[:], cw["wg"], gel[:],
                                               op0=ALU.mult, op1=ALU.mult)
                fft2[c].append(ft8)

        def ff_out(c, mos=(0, 2)):
            cs = slice(NC * c, NC * (c + 1))
            xT2 = lazy["xT2"]
            xT3 = lz("xT3", lambda: resid.tile([P, FT, S], F32, tag="x",
                                               name="xT3"))
            for mo in mos:
                pp = psA.tile([P, 2, NC], F32, tag="pA")
                for half in range(2):
                    m = mo + half
                    for j in range(NI // 2):
                        nc.tensor.matmul(pp[:, half, :],
                                         wfr[:, 2 * j:2 * j + 2,
                                             P * m:P * (m + 1)],
                                         fft2[c][j][:],
                                         start=(j == 0), stop=(j == NI // 2 - 1),
                                         perf_mode=PM.DoubleRow)
                    if flags["bf"]:
                        nc.tensor.matmul(pp[:, half, :],
                                         brow["bf"][:, P * m:P * (m + 1)],
                                         ones_row[:], start=False, stop=True,
                                         skip_group_check=True)
                nc.vector.scalar_tensor_tensor(
                    xT3[:, mo:mo + 2, cs], pp[:], cw["wf"] / SX,
                    xT2[:, mo:mo + 2, cs], op0=ALU.mult, op1=ALU.add)

        def store(c, cfunc=AF.Copy):
            xT3 = lazy["xT3"]
            for tp in range(2 * c, 2 * c + 2):
                pp = psA.tile([P, 2, NC], F32, tag="pA")
                for half in range(2):
                    stt = 2 * tp + half
                    for ft in range(FT):
                        nc.tensor.transpose(
                            pp[:, half, P * ft:P * (ft + 1)],
                            xT3[:, ft, P * stt:P * (stt + 1)], ident[:])
                for half in range(2):
                    ot = stage.tile([P, D], F32, tag="ot", bufs=3)
                    # ACT evac; func chosen to match the active act table
                    # (Identity = Ln/Exp table, Copy = gelu table)
                    nc.scalar.activation(ot[:], pp[:, half, :], cfunc)
                    nc.sync.dma_start(
                        out[P * (2 * tp + half):P * (2 * tp + half + 1), :],
                        ot[:])

        def dump(name, tile_ap):
            if dbg:
                nc.sync.dma_start(dbg_out[name][...], tile_ap)

        attn_self(qT1, kT1, v1sb, aT1, 0, [0], mid=emit_v_projs)
        for mo in range(0, FT, 2):
            project_pair(wq1r, "wq1", zv1, 1, mo, qT1, None,
                         "bq1", sd1.get(1), evac="dve")
        attn_self(qT1, kT1, v1sb, aT1, 0, [1, 2, 3])
        _enc_block()
        attn_self(qT1, kT1, v1sb, aT1, 1, [0, 1])
        pa1a(0)
        attn_self(qT1, kT1, v1sb, aT1, 1, [2])
        pa1b(0)
        attn_self(qT1, kT1, v1sb, aT1, 1, [3])
        pa1c(0)
        attn_cross(lazy["qT2"], 0, [0, 1])
        pa1a(1)
        attn_cross(lazy["qT2"], 0, [2, 3])
        pa1b(1)
        pa2a1(0)
        pa2a2(0)
        pa1c(1)
        # gelu c0 runs as one act-table-10 block; cross-attn c1 (table 6)
        # then overlaps the FF-c0 DVE/PE tail (ft8, wf matmuls, store)
        pa2b(0, [0, 1, 2, 3, 4, 5, 6, 7])
        attn_cross(lazy["qT2"], 1, [0, 1])
        attn_cross(lazy["qT2"], 1, [2, 3])
        pa2a1(1)
        ff_out(0)
        store(0, AF.Identity)
        pa2a2(1)
        pa2b(1, [0, 1, 2, 3, 4, 5, 6, 7])
        ff_out(1)
        store(1, AF.Copy)
        if dbg:
            dump("d_qT1", qT1[:])
            dump("d_kT1", kT1[:])
            dump("d_v1", v1sb[:])
            dump("d_aT1", aT1[:])
            dump("d_xT1", lazy["xT1"][:])
            dump("d_qT2", lazy["qT2"][:])
            dump("d_aT2", lazy["aT2"][:])
            dump("d_xT2", lazy["xT2"][:])
            dump("d_z3", lazy["z3"][:])
            dump("d_fft0", fft2[0][0][:])
            dump("d_kT2", kT2[:])
            dump("d_v2", v2sb[:])
            dump("d_encT", encT[:])

    return nc


_CACHED = {}


def _prep(inputs):
    """Host-side: fold LN gains, scale+convert weights to fp8, compute flags."""
    f32 = {k: np.ascontiguousarray(np.asarray(v), dtype=np.float32)
           for k, v in inputs.items()}
    g1, g2, g3 = f32["ln1_g"], f32["ln2_g"], f32["ln3_g"]
    b1, b2, b3 = f32["ln1_b"], f32["ln2_b"], f32["ln3_b"]
    wf32 = {
        "wq1": f32["wq1"] * g1[:, None], "wk1": f32["wk1"] * g1[:, None],
        "wv1": f32["wv1"] * g1[:, None], "wo1": f32["wo1"],
        "wq2": f32["wq2"] * g2[:, None], "wk2": f32["wk2"],
        "wv2": f32["wv2"], "wo2": f32["wo2"],
        "wg": f32["wg"] * g3[:, None], "wf": f32["wf"],
    }
    ws = {}
    w8 = {}
    for n, w in wf32.items():
        amax = float(np.abs(w).max()) or 1.0
        s = int(math.floor(math.log2(200.0 / amax)))
        ws[n] = s
        w8[n] = np.clip(w * (2.0 ** s), -240.0, 240.0).astype(
            ml_dtypes.float8_e4m3)
    bias = {
        "bq1": b1 @ f32["wq1"], "bk1": b1 @ f32["wk1"], "bv1": b1 @ f32["wv1"],
        "bq2": b2 @ f32["wq2"],
        "bo1": f32["bo1"], "bo2": f32["bo2"], "bf": f32["bf"],
        "bgu": b3 @ f32["wg"][:, :FF] + f32["bg"][:FF],
        "bgg": b3 @ f32["wg"][:, FF:] + f32["bg"][FF:],
    }
    flags = {n: bool(np.any(v != 0.0)) for n, v in bias.items()}
    dev_bias = {
        "bq1": bias["bq1"] * (SX * 2.0 ** ws["wq1"]),
        "bk1": bias["bk1"] * (SX * 2.0 ** ws["wk1"]),
        "bv1": bias["bv1"] * (SX * 2.0 ** ws["wv1"]),
        "bq2": bias["bq2"] * (SX * 2.0 ** ws["wq2"]),
        "bo1": bias["bo1"] * (SA * 2.0 ** ws["wo1"]),
        "bo2": bias["bo2"] * (SA * 2.0 ** ws["wo2"]),
        "bf": bias["bf"] * (SX * 2.0 ** ws["wf"]),
        "bgu": bias["bgu"] * (SX * 2.0 ** ws["wg"]),
        "bgg": bias["bgg"],    # true units (gelu bias)
    }
    return f32, w8, ws, dev_bias, flags


def _get_nc(key=None, ws=None, flags=None):
    if key is None:
        assert _CACHED, "kernel() must run before timeline queries"
        return next(iter(_CACHED.values()))
    if key not in _CACHED:
        nc = bacc.Bacc("TRN2", target_bir_lowering=False, debug=False,
                       num_devices=B)
        # bacc's act-table auto-pass maps Ln->natural_log and
        # Exp->exp_and_others (first table containing each func) and inserts
        # 15+ alternating table loads (~1.3us ACT each) across the
        # LN/softmax interleave. Feed it a doctored table list where only
        # natural_log_exp_and_others (Ln+Exp, one table for all of
        # layernorm+softmax) and gelu_and_others (Gelu+Copy, FF tail +
        # ACT store-copies) are matchable; emitted act_func_set_ids keep
        # their canonical positions so walrus' runtime remap is unchanged.
        import bass_rust as _bass_rust
        from concourse.hw_specs import get_activation_tables as _gat

        def _patched_insert_act_loads(_nc=nc):
            tabs = list(_gat(_nc.m.arch).items())
            doctored = []
            for name, funcs in tabs:
                if name == "natural_log_exp_and_others":
                    doctored.append((name, set(funcs) - {AF.Copy}))
                elif name == "gelu_and_others":
                    doctored.append((name, set(funcs)))
                else:
                    doctored.append((name, set()))
            _bass_rust.insert_act_table_loads(_nc, doctored)

        nc.insert_act_table_loads = _patched_insert_act_loads
        build(nc, ws, flags)
        nc.finalize()
        _CACHED[key] = nc
    return _CACHED[key]


def kernel(**inputs) -> np.ndarray:
    f32, w8, ws, dev_bias, flags = _prep(inputs)
    key = (tuple(sorted(flags.items())), tuple(sorted(ws.items())))
    nc = _get_nc(key, ws, flags)
    shared = dict(w8)
    for n, v in dev_bias.items():
        shared[n] = np.ascontiguousarray(v, dtype=np.float32)
    in_maps = [dict(shared, x=np.ascontiguousarray(f32["x"][i]),
                    enc=np.ascontiguousarray(f32["enc"][i])) for i in range(B)]
    res = run_bass_kernel_spmd(nc, in_maps, core_ids=list(range(B)))
    return np.stack([res.results[i]["out"] for i in range(B)], axis=0)


if __name__ == "__main__":
    print("module import ok")



# revision 65
# speedup vs baseline: 1.0486x; 1.0091x over previous
"""Trainium2 Bass kernel for a BasicTransformerBlock (self-attn + cross-attn + GEGLU FF).

Sharding: data-parallel over the batch axis - 8 batch elements onto 8 NeuronCores,
same SPMD program, no collectives.

v3 design (v2 + act-table + scheduling/balance work; 247us -> 233us):
- Heavy matmuls run fp8e4 (TRN e4m3) with DoubleRow perf mode (0.5 cyc/row, K=256
  per instruction). Weights are power-of-2-scaled + converted to fp8 on the HOST
  and DMA'd straight into SBUF; QKV1 weights DMA first so LN1->projections are
  never input-starved.
- Residual stream stays transposed f32r xT[d, s]. LayerNorm gain g folds into
  the weights host-side. 1/std folds ONCE into zv1 = (x-mu)*SX*rstd (column
  scaling commutes through feature-mixing matmuls), so q/k/v all project from
  zv1 and their PSUM->SBUF evacuations are pure scales on the (then-idle) ACT
  engine as Identity ops. x psum->sbuf restage also runs on ACT at load time.
- Softmax: scores land in 2-bank PSUM pairs, one ACT exp per pair writing fp8
  probs 16*e^s in DoubleRow [p,2,n] layout; denominators via ones-lhsT
  DoubleRow matmuls. aT stays head-dim-major [64, NH, S] (walrus rejects any
  nonzero matmul psum dst partition); normalization = one DVE reciprocal +
  Pool partition-broadcasts + one fused stt per (c, hf).
- Activation tables: bacc's auto-pass is fed a doctored table list where only
  natural_log_exp_and_others (Ln/Exp/Identity) and gelu_and_others (Gelu/Copy)
  are matchable, collapsing 19 alternating table loads (~24us ACT) to 2-3.
  Gelus run as contiguous per-chunk blocks; ACT copies use Identity in the
  Ln/Exp phase and Copy in the gelu phase.
- Scales: activations x16, probs x16, attention out x64, per-weight 2^s with
  s = floor(log2(200/absmax)).
- Software pipelining: attention tails deferred one hf; encoder K/V block
  emitted inside the self-attention window; GEGLU chunk 0 emitted before
  cross-attention chunk 1 so the FF-c0 DVE/PE tail overlaps the thin
  cross-attn latency chains; FF psum g-tiles alternate between the psP slot
  and the stats slot (double-buffering); LN xsq and zv multiplies split
  across DVE and Pool. The LN mu-broadcast psum and the mo==0 / even-tp
  projection psums rotate through the 1-buf stats slot instead of the score
  slots, so attention score matmuls (which feed the ACT-critical exp stream)
  are never starved of psum during the LN2/LN3/v-proj windows.
"""
import sys

sys.path.insert(0, "/opt/trn_rl_repo")

import math
from contextlib import ExitStack

import ml_dtypes
import numpy as np

import concourse.bass as bass
import concourse.mybir as mybir
import concourse.tile as tile
from concourse import bacc
from concourse.bass_utils import run_bass_kernel_spmd
from concourse.masks import make_identity

F32 = mybir.dt.float32
F32R = mybir.dt.float32r
FP8 = mybir.dt.float8e4
AF = mybir.ActivationFunctionType
ALU = mybir.AluOpType
PM = mybir.MatmulPerfMode

B = 8
S = 1024          # tokens
D = 512           # model dim
SK2 = 77          # cross-attention source length
DE = 768          # encoder dim
FF = 2048         # GEGLU inner dim (per half)
NH = 8            # heads
DH = 64           # head dim
DHP = 80          # padded head slot in v tiles (16B-aligned pair strides);
                  # col 64 holds the ones column that folds the softmax
                  # denominator into the pv matmul (M=65, den = pv[64])
SCALE = DH ** -0.5
EPS = 1e-5
P = 128
NC = 512          # token chunk (one psum bank of fp32)
ST = S // P       # 8 token tiles
FT = D // P       # 4 feature tiles
CH = S // NC      # 2 token chunks
KE = DE // P      # 6 encoder feature tiles
NI = FF // P      # 16 FF inner tiles

SX = 16.0         # activation scale (z, enc, q, k, v)
SEXP = 16.0       # probs scale
SA = 64.0         # attention-output scale
LN_SEXP = math.log(SEXP)

WNAMES = ["wq1", "wk1", "wv1", "wo1", "wq2", "wk2", "wv2", "wo2", "wg", "wf"]


def build(nc: bass.Bass, ws: dict, flags: dict, dbg=False):
    """ws: name -> log2 weight scale. flags: name -> bias vector is nonzero."""
    x = nc.dram_tensor("x", [S, D], F32, kind="ExternalInput")
    enc = nc.dram_tensor("enc", [SK2, DE], F32, kind="ExternalInput")
    w_in = {}
    for name, shape in [
        ("wq1", [D, D]), ("wk1", [D, D]), ("wv1", [D, D]), ("wo1", [D, D]),
        ("wq2", [D, D]), ("wk2", [DE, D]), ("wv2", [DE, D]), ("wo2", [D, D]),
        ("wg", [D, 2 * FF]), ("wf", [FF, D]),
    ]:
        w_in[name] = nc.dram_tensor(name, shape, FP8, kind="ExternalInput")
    vec_in = {}
    for name, n in [("bq1", D), ("bk1", D), ("bv1", D), ("bq2", D),
                    ("bo1", D), ("bo2", D), ("bf", D),
                    ("bgu", FF), ("bgg", FF)]:
        vec_in[name] = nc.dram_tensor(name, [n], F32, kind="ExternalInput")
    out = nc.dram_tensor("out", [S, D], F32, kind="ExternalOutput")
    dbg_out = {}
    if dbg:
        for name, shape, dt in [
            ("d_qT1", [P, FT, S], FP8), ("d_kT1", [P, FT, S], FP8),
            ("d_v1", [P, ST // 2, 2, NH, DH], FP8),
            ("d_aT1", [DH, NH, S], FP8), ("d_xT1", [P, FT, S], F32R),
            ("d_qT2", [P, FT, S], FP8), ("d_aT2", [DH, NH, S], FP8),
            ("d_xT2", [P, FT, S], F32R), ("d_z3", [P, FT, S], FP8),
            ("d_fft0", [P, 2, NC], FP8), ("d_kT2", [P, FT, 80], FP8),
            ("d_rc", [1, 2, NC], F32), ("d_bc", [DH, 2, NC], F32),
            ("d_v2", [SK2, NH, DH], FP8), ("d_encT", [P, KE, 80], FP8),
        ]:
            dbg_out[name] = nc.dram_tensor(name, shape, dt,
                                           kind="ExternalOutput")

    cw = {n: 2.0 ** -ws[n] for n in WNAMES}   # descale constants

    with tile.TileContext(nc) as tc, ExitStack() as es:
        const = es.enter_context(tc.tile_pool(name="const", bufs=1))
        resid = es.enter_context(tc.tile_pool(name="resid", bufs=2))
        zp = es.enter_context(tc.tile_pool(name="zp", bufs=2))
        att = es.enter_context(tc.tile_pool(name="att", bufs=1))
        wp = es.enter_context(tc.tile_pool(name="wp", bufs=1))
        stage = es.enter_context(tc.tile_pool(name="stage", bufs=2))
        t3p = es.enter_context(tc.tile_pool(name="t3p", bufs=1))
        rowp = es.enter_context(tc.tile_pool(name="rowp", bufs=1))
        bcp = es.enter_context(tc.tile_pool(name="bcp", bufs=3))
        exp_p = es.enter_context(tc.tile_pool(name="exp", bufs=3))
        gelp = es.enter_context(tc.tile_pool(name="gelp", bufs=2))
        fftp = es.enter_context(tc.tile_pool(name="fftp", bufs=8))
        psA = es.enter_context(tc.tile_pool(name="psA", bufs=2, space="PSUM"))
        psP = es.enter_context(tc.tile_pool(name="psP", bufs=1, space="PSUM"))

        # ---- constants ----
        ident = const.tile([P, P], F32, tag="ident")
        make_identity(nc, ident[:])
        ones_f = const.tile([P, P], F32, tag="ones_f")
        nc.vector.memset(ones_f[:], 1.0)
        ones128 = const.tile([P, 1], F32R, tag="o128")  # stats lhsT (K=128,M=1)
        nc.vector.tensor_copy(ones128[:], ones_f[:, 0:1])
        ones1x = const.tile([1, P], F32R, tag="o1x")    # bcast lhsT (K=1)
        nc.vector.tensor_copy(ones1x[:], ones_f[0:1, :])
        ones_row = None
        if any(flags.values()):
            ones_row_f = const.tile([1, NC], F32, tag="orowf")
            nc.vector.memset(ones_row_f[:], 1.0)
            ones_row = const.tile([1, NC], F32R, tag="orow")  # rank-1 bias rhs
            nc.vector.tensor_copy(ones_row[:], ones_row_f[:])
        ones8 = const.tile([P, 2, 16], FP8, tag="o8")   # self denominator lhsT
        nc.vector.memset(ones8[:], 1.0)                 # (16B-aligned pair stride)
        ones2 = const.tile([SK2, 1], FP8, tag="o2")     # cross denominator lhsT
        nc.vector.memset(ones2[:], 1.0)
        lnsc = const.tile([P, 1], F32, tag="lnsc")      # exp bias = ln(SEXP)
        nc.vector.memset(lnsc[:], LN_SEXP)
        eps_t = const.tile([1, 1], F32, tag="eps")
        nc.vector.memset(eps_t[:], EPS)
        # explicit activation-table management: one Ln+Exp table for all of
        # layernorm + softmax, a single switch to the gelu table for the FF
        # tail (Copy is in both tables, so ACT evacuations never force a
        # switch). bacc's auto-insertion pass is disabled in _get_nc — it
        # maps Ln->natural_log and Exp->exp_and_others and thrashes 15+
        # table loads (~1.3us ACT each) across the LN/softmax interleave.
        try:
            from concourse.hw_specs import get_activation_tables
            _t = list(get_activation_tables(nc.m.arch))
            TAB_LNEXP = _t.index("natural_log_exp_and_others")
            TAB_GELU = _t.index("gelu_and_others")
        except Exception:
            TAB_LNEXP, TAB_GELU = 6, 10

        def load_table(tid):
            if tid is not None:
                nc.scalar.add_instruction(mybir.InstLoadActFuncSet(
                    name=nc.get_next_instruction_name(), ins=[], outs=[],
                    act_func_set_id=tid))

        warm = const.tile([1, 1], F32, tag="warm")
        nc.scalar.activation(warm[:], eps_t[:], AF.Ln)   # pull act-table load
        nc.scalar.activation(warm[:], warm[:], AF.Exp)   # into the DMA window

        # ---- weights: DMA fp8 straight into SBUF ----
        def load_w(name, kouter):
            dram = w_in[name]
            wr = wp.tile([P, kouter, dram.shape[1]], FP8, tag=f"{name}r")
            nc.sync.dma_start(wr[:], dram.rearrange("(ko ki) n -> ki ko n", ki=P))
            return wr

        bgg_c = const.tile([P, NI], F32, tag="bgg_c")
        if flags["bgg"]:
            nc.sync.dma_start(bgg_c[:],
                              vec_in["bgg"].rearrange("(o p) -> p o", p=P))

        def bias_row(name):
            """[1, N] f32r row of host-scaled bias, for rank-1 lhsT slices."""
            n = vec_in[name].shape[0]
            tf = const.tile([1, n], F32, tag=f"{name}_f")
            nc.sync.dma_start(tf[:], vec_in[name].rearrange("(o n) -> o n", o=1))
            tr = const.tile([1, n], F32R, tag=f"{name}_r")
            nc.vector.tensor_copy(tr[:], tf[:])
            return tr

        brow = {name: bias_row(name)
                for name in ("bq1", "bk1", "bv1", "bq2", "bo1", "bo2", "bf",
                             "bgu") if flags[name]}

        # ---- load x, transpose into xT [128, FT, S] (f32r) ----
        # x and enc DMAs go first so the transposes are never input-starved;
        # weight DMAs queue behind them and overlap LN1/QKV1 compute.
        xT = resid.tile([P, FT, S], F32R, tag="x")
        x_stage = []
        for st in range(ST):
            xr = stage.tile([P, D], F32, tag="x_raw", bufs=3)
            nc.sync.dma_start(xr[:], x[P * st:P * (st + 1), :])
            x_stage.append(xr)
        def load_wo(name):
            dram = w_in[name]
            wr = wp.tile([DH, NH, dram.shape[1]], FP8, tag=f"{name}r")
            nc.sync.dma_start(wr[:], dram.rearrange("(h k) n -> k h n", k=DH))
            return wr

        wq1r = load_w("wq1", FT)
        wk1r = load_w("wk1", FT)
        wv1r = load_w("wv1", FT)
        enc_raw = const.tile([SK2, DE], F32, tag="enc_raw")
        nc.sync.dma_start(enc_raw[:], enc[:, :])
        wo1r = load_wo("wo1")
        wq2r = load_w("wq2", FT)
        wk2r = load_w("wk2", KE)
        wv2r = load_w("wv2", KE)
        wo2r = load_wo("wo2")
        wgr = load_w("wg", FT)
        wfr = load_w("wf", NI)
        def load_x_pair(tp):
            pt = psA.tile([P, 2, NC], F32, tag="pA")
            for half in range(2):
                st = 2 * tp + half
                xr = x_stage[st]
                for ft in range(FT):
                    nc.tensor.transpose(pt[:, half, P * ft:P * (ft + 1)],
                                        xr[:, P * ft:P * (ft + 1)], ident[:])
            # psum->xT restage on ACT (idle during the load phase;
            # Identity is in the Ln/Exp table so no act-table switch)
            nc.scalar.activation(
                xT[:, :, 2 * P * tp:2 * P * (tp + 1)].rearrange(
                    "p f (a q) -> p a f q", a=2),
                pt[:].rearrange("p a (f q) -> p a f q", f=FT), AF.Identity)

        # ---- enc -> encT [128, KE, 80] fp8 (x16), cols 77:80 zero-padded
        # (pair slices need 16B-aligned outer stride for dual-fp8 ldweights).
        # Emitted later, during ACT-bound self-attention: only cross-attn
        # consumes these, and their DVE copies would otherwise delay LN1. ----
        SK2P = 80
        SK2A = 80   # padded so [hp, hf, :] slices stay 4B-aligned
        encT = att.tile([P, KE, SK2P], FP8, tag="encT")
        kT2 = att.tile([P, FT, SK2A], FP8, tag="kT2")
        v2sb = att.tile([SK2, NH, DH], FP8, tag="v2sb")

        def emit_enc_block():
            pass

        def _enc_block():
            nc.vector.memset(encT[:, :, SK2:SK2P], 0.0)
            for kp in range(KE // 2):
                pt = psA.tile([P, 2, NC], F32, tag="pA")
                for half in range(2):
                    ke = 2 * kp + half
                    nc.tensor.transpose(pt[:, half, 0:SK2],
                                        enc_raw[:, P * ke:P * (ke + 1)],
                                        ident[0:SK2, 0:SK2])
                nc.vector.tensor_scalar_mul(
                    encT[:, 2 * kp:2 * kp + 2, 0:SK2], pt[:, :, 0:SK2], SX)
            for mp in range(FT // 2):
                pt = psA.tile([P, 2, NC], F32, tag="pA")
                for half in range(2):
                    mo = 2 * mp + half
                    for t in range(KE // 2):
                        nc.tensor.matmul(pt[:, half, 0:SK2P],
                                         wk2r[:, 2 * t:2 * t + 2,
                                              P * mo:P * (mo + 1)],
                                         encT[:, 2 * t:2 * t + 2, :],
                                         start=(t == 0),
                                         stop=(t == KE // 2 - 1),
                                         perf_mode=PM.DoubleRow)
                nc.vector.tensor_scalar_mul(kT2[:, 2 * mp:2 * mp + 2, 0:SK2],
                                            pt[:, :, 0:SK2], cw["wk2"])
            ptv = psA.tile([P, 2, NC], F32, tag="pA", name="ptv2")
            for t in range(KE // 2):
                nc.tensor.matmul(ptv[0:SK2P, 0, :],
                                 encT[:, 2 * t:2 * t + 2, :],
                                 wv2r[:, 2 * t:2 * t + 2, :],
                                 start=(t == 0), stop=(t == KE // 2 - 1),
                                 perf_mode=PM.DoubleRow)
            nc.vector.tensor_scalar_mul(
                v2sb[:].rearrange("s h d -> s (h d)"), ptv[0:SK2, 0, :],
                cw["wv2"])

        # ---- LayerNorm: stats + rows; z via gpsimd (all-SBUF) ----
        def ln_sums(src, c, tagpfx):
            cs = slice(NC * c, NC * (c + 1))
            sum_ps = psA.tile([1, 2, NC], F32, tag="den", bufs=1,
                              name=f"st_{tagpfx}{c}")
            for ft in range(FT):
                nc.tensor.matmul(sum_ps[:, 0, :], ones128[:], src[:, ft, cs],
                                 start=(ft == 0), stop=(ft == FT - 1))
            for ft in range(FT):
                xsq = stage.tile([P, NC], F32R, tag="xsq", bufs=3)
                sq_eng = nc.vector if (tagpfx == "l1" or ft < 2) else nc.gpsimd
                sq_eng.tensor_tensor(xsq[:], src[:, ft, cs], src[:, ft, cs],
                                     ALU.mult)
                nc.tensor.matmul(sum_ps[:, 1, :], ones128[:], xsq[:],
                                 start=(ft == 0), stop=(ft == FT - 1),
                                 skip_group_check=True)
            return sum_ps

        def ln_stats(src, c, tagpfx, rstd_dtype=F32, sum_ps=None):
            """-> (mu16_sb [P,NC], rstd row, lnv row) for chunk c of src."""
            if sum_ps is None:
                sum_ps = ln_sums(src, c, tagpfx)
            mu = rowp.tile([1, NC], F32, tag="mu", bufs=2)
            nc.vector.tensor_scalar_mul(mu[:], sum_ps[:, 0, :], 1.0 / D)
            mu16 = rowp.tile([1, NC], F32R, tag="mu16")
            nc.vector.tensor_scalar_mul(mu16[:], mu[:], SX)
            musq = rowp.tile([1, NC], F32, tag="musq")
            nc.gpsimd.tensor_tensor(musq[:], mu[:], mu[:], ALU.mult)
            var = rowp.tile([1, NC], F32, tag="var", bufs=2)
            nc.vector.scalar_tensor_tensor(var[:], sum_ps[:, 1, :], 1.0 / D,
                                           musq[:], op0=ALU.mult,
                                           op1=ALU.subtract)
            lnv = rowp.tile([1, NC], F32, tag="lnv", bufs=2)
            nc.scalar.activation(lnv[:], var[:], AF.Ln, bias=eps_t[:])
            rstd = rowp.tile([1, NC], rstd_dtype, tag="rstd", bufs=2)
            nc.scalar.activation(rstd[:], lnv[:], AF.Exp, scale=-0.5)
            # rotate through the stats slot rather than a score slot so
            # attention score matmuls aren't starved during LN2/LN3
            mbt = psA.tile([P, 2, NC], F32, tag="den", bufs=1,
                           name=f"mub_{tagpfx}{c}")
            nc.tensor.matmul(mbt[:, 0, :], ones1x[:], mu16[:], start=True,
                             stop=True)
            return mbt[:, 0:1, :], rstd, lnv

        def z_from(src, mu16_b, c, zt):
            cs = slice(NC * c, NC * (c + 1))
            for mo in range(0, FT, 2):
                nc.vector.scalar_tensor_tensor(
                    zt[:, mo:mo + 2, cs], src[:, mo:mo + 2, cs], SX,
                    mu16_b.broadcast_to([P, 2, NC]),
                    op0=ALU.mult, op1=ALU.subtract)

        def sd_row_from(lnv):
            sd = rowp.tile([1, NC], F32R, tag="sd", bufs=2)
            nc.scalar.activation(sd[:], lnv[:], AF.Exp, scale=0.5)
            return sd

        # ---- projection helpers ----
        def project_pair(w_r, wname, zt, c, mo, dst, rstd_sb=None, bname=None,
                         sd_row=None, evac="act"):
            """dst[:, mo:mo+2, cs] = (W.T@z * cw) [* rstd] (+ bias via rank-1).
            evac: 'act' = ACT Identity (pure scale; zt must carry rstd
            already), 'dve' = DVE tensor_scalar (pure scale), 'dve_rstd' =
            DVE stt with per-token rstd (zt un-rstd'd)."""
            cs = slice(NC * c, NC * (c + 1))
            pp = psA.tile([P, 2, NC], F32, tag="den" if mo == 0 else "pA",
                          bufs=1 if mo == 0 else 2)
            for half in range(2):
                m = mo + half
                for t in range(FT // 2):
                    nc.tensor.matmul(pp[:, half, :],
                                     w_r[:, 2 * t:2 * t + 2, P * m:P * (m + 1)],
                                     zt[:, 2 * t:2 * t + 2, cs],
                                     start=(t == 0), stop=(t == FT // 2 - 1),
                                     perf_mode=PM.DoubleRow)
                if bname is not None and bname in brow:
                    nc.tensor.matmul(pp[:, half, :],
                                     brow[bname][:, P * m:P * (m + 1)],
                                     sd_row[:], start=False, stop=True,
                                     skip_group_check=True)
            if evac == "act":
                nc.scalar.activation(dst[:, mo:mo + 2, cs], pp[:],
                                     AF.Identity, scale=cw[wname])
            elif evac == "dve":
                nc.vector.tensor_scalar_mul(dst[:, mo:mo + 2, cs], pp[:],
                                            cw[wname])
            else:
                nc.vector.scalar_tensor_tensor(
                    dst[:, mo:mo + 2, cs], pp[:], cw[wname],
                    rstd_sb[:, None, :].broadcast_to([P, 2, NC]),
                    op0=ALU.mult, op1=ALU.mult)

        def out_proj_pair(w_r, wname, aT, c, mo, src, dst, post, bname=None):
            """dst[:, mo:mo+2, cs] = src + (W.T@a * cw * post) [+ bias].
            aT is head-dim-major [DH, NH, S]; contraction runs over head
            pairs (K = 2*DH per DoubleRow matmul)."""
            cs = slice(NC * c, NC * (c + 1))
            pp = psA.tile([P, 2, NC], F32, tag="den" if mo == 0 else "pA",
                          bufs=1 if mo == 0 else 2)
            for half in range(2):
                m = mo + half
                for t in range(NH // 2):
                    nc.tensor.matmul(pp[:, half, :],
                                     w_r[:, 2 * t:2 * t + 2, P * m:P * (m + 1)],
                                     aT[:, 2 * t:2 * t + 2, cs],
                                     start=(t == 0), stop=(t == NH // 2 - 1),
                                     perf_mode=PM.DoubleRow)
                if bname is not None and bname in brow:
                    nc.tensor.matmul(pp[:, half, :],
                                     brow[bname][:, P * m:P * (m + 1)],
                                     ones_row[:], start=False, stop=True,
                                     skip_group_check=True)
            nc.vector.scalar_tensor_tensor(
                dst[:, mo:mo + 2, cs], pp[:], cw[wname] * post,
                src[:, mo:mo + 2, cs], op0=ALU.mult, op1=ALU.add)

        # ---- attention (self) for one chunk ----
        # rc_act: cross-attn tails land in the (DVE-bound, ACT-idle) FF
        # window - compute 1/den as Exp(-Ln(den)) on ACT there instead of
        # a DVE reciprocal. Self-attn tails run while ACT is exp-saturated,
        # so they keep the DVE reciprocal.
        def attn_tail(aT, c, hf, pv, den):
            cs = slice(NC * c, NC * (c + 1))
            rc = rowp.tile([1, 2, NC], F32, tag="rc", bufs=2)
            nc.vector.reciprocal(rc[:], den[:])
            bc = bcp.tile([DH, 2, NC], F32, tag="bc", bufs=3)
            nc.gpsimd.partition_broadcast(bc[:, 0, :], rc[:, 0, :])
            nc.gpsimd.partition_broadcast(bc[:, 1, :], rc[:, 1, :])
            nc.vector.scalar_tensor_tensor(
                aT[:, 2 * hf:2 * hf + 2, cs], pv[:], SA / SEXP,
                bc[:], op0=ALU.mult, op1=ALU.mult)

        pend = []   # software-pipelined attention tails across calls

        def flush_tail():
            if pend:
                attn_tail(*pend.pop())

        def attn_self(qT, kT, v1sb, aT, c, hfs, mid=None):
            cs = slice(NC * c, NC * (c + 1))
            for hf in hfs:
                exs = []
                for par in range(2):
                    hp = slice(DH * par, DH * par + DH)
                    for j in range(ST // 2):
                        if mid is not None and hf == hfs[0]:
                            if par == 0 and j == 2:
                                mid(0)
                            elif par == 1 and j == 0:
                                mid(1)
                        sc = psA.tile([P, 2, NC], F32, tag="pA")
                        for half in range(2):
                            sk = 2 * j + half
                            nc.tensor.matmul(
                                sc[:, half, :],
                                kT[hp, hf, P * sk:P * (sk + 1)],
                                qT[hp, hf, cs], start=True, stop=True)
                        ex = exp_p.tile([P, 2, NC], FP8, tag="ex", bufs=6)
                        nc.scalar.activation(ex[:], sc[:], AF.Exp,
                                             scale=SCALE / (SX * SX),
                                             bias=lnsc[:])
                        exs.append(ex)
                flush_tail()
                pv = psP.tile([DH, 2, NC], F32, tag="pvp",
                              name=f"pv1_{c}_{hf}")
                den = psA.tile([1, 2, NC], F32, tag="den", bufs=1,
                               name=f"den1_{c}_{hf}")
                for par in range(2):
                    h = 2 * hf + par
                    for j in range(ST // 2):
                        ex = exs[par * (ST // 2) + j]
                        nc.tensor.matmul(pv[:, par, :],
                                         v1sb[:, j, :, h, :], ex[:],
                                         start=(j == 0),
                                         stop=(j == ST // 2 - 1),
                                         perf_mode=PM.DoubleRow,
                                         skip_group_check=True)
                        nc.tensor.matmul(den[:, par, :], ones8[:, :, 0:1], ex[:],
                                         start=(j == 0), stop=(j == ST // 2 - 1),
                                         perf_mode=PM.DoubleRow,
                                         skip_group_check=True)
                pend.append((aT, c, hf, pv, den))

        # ---- attention (cross) for one chunk ----
        def attn_cross(qT, c, hfs):
            aT = lz("aT2", lambda: att.tile([DH, NH, S], FP8, tag="ka",
                                            bufs=1, name="aT2"))
            cs = slice(NC * c, NC * (c + 1))
            for hf in hfs:
                sc = psA.tile([P, 2, NC], F32, tag="pA")
                for par in range(2):
                    hp = slice(DH * par, DH * par + DH)
                    nc.tensor.matmul(sc[0:SK2, par, :], kT2[hp, hf, 0:SK2],
                                     qT[hp, hf, cs], start=True, stop=True)
                ex = exp_p.tile([SK2, 2, NC], FP8, tag="ex2c", bufs=2)
                nc.scalar.activation(ex[:], sc[0:SK2, :, :], AF.Exp,
                                     scale=SCALE / (SX * SX),
                                     bias=lnsc[0:SK2, :])
                flush_tail()
                pv = psP.tile([DH, 2, NC], F32, tag="pvp",
                              name=f"pv2_{c}_{hf}")
                den = psA.tile([1, 2, NC], F32, tag="den", bufs=1,
                               name=f"den2_{c}_{hf}")
                for par in range(2):
                    h = 2 * hf + par
                    nc.tensor.matmul(pv[:, par, :],
                                     v2sb[:, h, :], ex[:, par, :],
                                     start=True, stop=True,
                                     skip_group_check=True)
                    nc.tensor.matmul(den[:, par, :], ones2[:], ex[:, par, :],
                                     start=True, stop=True,
                                     skip_group_check=True)
                pend.append((aT, c, hf, pv, den))

        # ================= LN1 + QKV (both chunks) =================
        # rstd folds into zv1 once (column scaling commutes through the
        # feature-mixing projections), so q/k/v all project from zv1 and
        # their evacuations become pure scales (ACT Identity / DVE scale)
        zv1 = zp.tile([P, FT, S], FP8, tag="z")
        qT1 = att.tile([P, FT, S], FP8, tag="qt", bufs=1)
        kT1 = att.tile([P, FT, S], FP8, tag="ka", bufs=1)
        v1sb = att.tile([P, ST // 2, 2, NH, DH], FP8, tag="v1sb")
        rstd1_sb = {}
        sd1 = {}
        need_sd1 = flags["bq1"] or flags["bk1"] or flags["bv1"]
        def v_proj_pair(c, tp):
            pp = psA.tile([P, 2, NC], F32, tag="den" if tp % 2 == 0 else "pA",
                          bufs=1 if tp % 2 == 0 else 2)
            for half in range(2):
                stt = 2 * tp + half
                for t in range(FT // 2):
                    nc.tensor.matmul(
                        pp[:, half, :],
                        zv1[:, 2 * t:2 * t + 2, P * stt:P * (stt + 1)],
                        wv1r[:, 2 * t:2 * t + 2, :],
                        start=(t == 0), stop=(t == FT // 2 - 1),
                        perf_mode=PM.DoubleRow)
                if flags["bv1"]:
                    off = P * stt - NC * c
                    nc.tensor.matmul(
                        pp[:, half, :],
                        sd1[c][:, off:off + P], brow["bv1"][:],
                        start=False, stop=True, skip_group_check=True)
            nc.vector.tensor_scalar_mul(
                v1sb[:, tp, :, :, :].rearrange("p a h d -> p (a h d)"),
                pp[:].rearrange("p a n -> p (a n)"), cw["wv1"])

        for tp in range(ST // 2):
            load_x_pair(tp)
        mu16b1 = {}
        for c in range(CH):
            cs = slice(NC * c, NC * (c + 1))
            mu16_b, rstd, lnv = ln_stats(xT, c, "l1")
            mu16b1[c] = mu16_b
            rsb = bcp.tile([P, NC], F32, tag="rstdb", name=f"rstd1b_{c}")
            nc.gpsimd.partition_broadcast(rsb[:], rstd[:])
            rstd1_sb[c] = rsb
            if need_sd1:
                sd1[c] = sd_row_from(lnv)
            for mo in range(0, FT, 2):
                t1 = t3p.tile([P, 2, NC], F32, tag="t1", bufs=2)
                nc.vector.scalar_tensor_tensor(
                    t1[:], xT[:, mo:mo + 2, cs], SX,
                    mu16_b.broadcast_to([P, 2, NC]),
                    op0=ALU.mult, op1=ALU.subtract)
                zv_eng = nc.vector if mo == 0 else nc.gpsimd
                zv_eng.tensor_tensor(
                    zv1[:, mo:mo + 2, cs], t1[:],
                    rsb[:, None, :].broadcast_to([P, 2, NC]), ALU.mult)
        # K projections first (attention needs all keys), then q(c0)/v, then
        # q(c1) is deferred into the first attention call
        for c in range(CH):
            for mo in range(0, FT, 2):
                project_pair(wk1r, "wk1", zv1, c, mo, kT1, None,
                             "bk1", sd1.get(c), evac="act")
        for mo in range(0, FT, 2):
            project_pair(wq1r, "wq1", zv1, 0, mo, qT1, None,
                         "bq1", sd1.get(0), evac="act")

        def emit_v_projs(half):
            for tp in (2 * half, 2 * half + 1):
                v_proj_pair(tp // 2, tp)

        # ================= chunk-pipelined main pass =================
        # tiles are allocated lazily at first use so the resid/z/ka tag
        # rotations only reclaim slots whose previous tenant is dead
        aT1 = att.tile([DH, NH, S], FP8, tag="aT1")
        lazy = {}

        def lz(name, ctor):
            if name not in lazy:
                lazy[name] = ctor()
            return lazy[name]

        rstd2_sb = {}
        sd2 = {}
        fft2 = {0: [], 1: []}

        def pa1a(c):
            flush_tail()
            xT1 = lz("xT1", lambda: resid.tile([P, FT, S], F32R, tag="x",
                                               name="xT1"))
            for mo in range(0, FT, 2):
                out_proj_pair(wo1r, "wo1", aT1, c, mo, xT, xT1, 1.0 / SA, "bo1")

        def pa1b(c):
            xT1 = lazy["xT1"]
            mu16_sb, rstd, lnv = ln_stats(xT1, c, "l2")
            rsb = bcp.tile([P, NC], F32, tag="rstdb", name=f"rstd2b_{c}")
            nc.gpsimd.partition_broadcast(rsb[:], rstd[:])
            rstd2_sb[c] = rsb
            if flags["bq2"]:
                sd2[c] = sd_row_from(lnv)
            z2 = lz("z2", lambda: zp.tile([P, FT, S], FP8, tag="z",
                                          name="z2"))
            z_from(xT1, mu16_sb, c, z2)

        def pa1c(c):
            z2 = lazy["z2"]
            qT2 = lz("qT2", lambda: att.tile([P, FT, S], FP8, tag="qt",
                                             bufs=1, name="qT2"))
            for mo in range(0, FT, 2):
                project_pair(wq2r, "wq2", z2, c, mo, qT2, rstd2_sb[c],
                             "bq2", sd2.get(c), evac="dve_rstd")

        def pa2a1(c):
            flush_tail()
            xT1 = lazy["xT1"]
            aT2 = lazy["aT2"]
            xT2 = lz("xT2", lambda: resid.tile([P, FT, S], F32R, tag="x",
                                               name="xT2"))
            for mo in range(0, FT, 2):
                out_proj_pair(wo2r, "wo2", aT2, c, mo, xT1, xT2, 1.0 / SA, "bo2")

        def pa2a2(c):
            cs = slice(NC * c, NC * (c + 1))
            xT2 = lazy["xT2"]
            z3 = lz("z3", lambda: zp.tile([P, FT, S], FP8, tag="z", name="z3"))
            mu16_sb, rstd, lnv = ln_stats(xT2, c, "l3")
            r3b = bcp.tile([P, NC], F32, tag="rstdb", name=f"rstd3b_{c}")
            nc.gpsimd.partition_broadcast(r3b[:], rstd[:])
            for mo in range(0, FT, 2):
                t3 = t3p.tile([P, 2, NC], F32, tag="t1", bufs=2)
                nc.vector.scalar_tensor_tensor(
                    t3[:], xT2[:, mo:mo + 2, cs], SX,
                    mu16_sb.broadcast_to([P, 2, NC]),
                    op0=ALU.mult, op1=ALU.subtract)
                nc.gpsimd.tensor_tensor(
                    z3[:, mo:mo + 2, cs], t3[:],
                    r3b[:, None, :].broadcast_to([P, 2, NC]), ALU.mult)

        def pa2b(c, js):
            # GEGLU inner: g/u pair tiles -> gelu -> fft (fp8 DoubleRow layout)
            cs = slice(NC * c, NC * (c + 1))
            z3 = lazy["z3"]
            for j in js:
                if j % 2 == 0:
                    pg = psP.tile([P, 2, NC], F32, tag="pvp",
                                  name=f"pg_{c}_{j}")
                else:
                    pg = psA.tile([P, 2, NC], F32, tag="den", bufs=1,
                                  name=f"pg_{c}_{j}")
                pu = psA.tile([P, 2, NC], F32, tag="pA")
                for half in range(2):
                    i = 2 * j + half
                    for t in range(FT // 2):
                        nc.tensor.matmul(pg[:, half, :],
                                         wgr[:, 2 * t:2 * t + 2,
                                             FF + P * i:FF + P * (i + 1)],
                                         z3[:, 2 * t:2 * t + 2, cs],
                                         start=(t == 0), stop=(t == FT // 2 - 1),
                                         perf_mode=PM.DoubleRow)
                for half in range(2):
                    i = 2 * j + half
                    for t in range(FT // 2):
                        nc.tensor.matmul(pu[:, half, :],
                                         wgr[:, 2 * t:2 * t + 2,
                                             P * i:P * (i + 1)],
                                         z3[:, 2 * t:2 * t + 2, cs],
                                         start=(t == 0), stop=(t == FT // 2 - 1),
                                         perf_mode=PM.DoubleRow)
                    if flags["bgu"]:
                        nc.tensor.matmul(pu[:, half, :],
                                         brow["bgu"][:, P * i:P * (i + 1)],
                                         ones_row[:], start=False, stop=True,
                                         skip_group_check=True)
                gel = gelp.tile([P, 2, NC], FP8, tag="gel", bufs=3)
                if flags["bgg"]:
                    for half in range(2):
                        i = 2 * j + half
                        nc.scalar.activation(gel[:, half, :], pg[:, half, :],
                                             AF.Gelu, scale=cw["wg"] / SX,
                                             bias=bgg_c[:, i:i + 1])
                else:
                    nc.scalar.activation(gel[:], pg[:], AF.Gelu,
                                         scale=cw["wg"] / SX)
                ft8 = fftp.tile([P, 2, NC], FP8, tag="ft8")
                nc.vector.scalar_tensor_tensor(ft8[:], pu[:], cw["wg"], gel[:],
                                               op0=ALU.mult, op1=ALU.mult)
                fft2[c].append(ft8)

        def ff_out(c, mos=(0, 2)):
            cs = slice(NC * c, NC * (c + 1))
            xT2 = lazy["xT2"]
            xT3 = lz("xT3", lambda: resid.tile([P, FT, S], F32, tag="x",
                                               name="xT3"))
            for mo in mos:
                pp = psA.tile([P, 2, NC], F32, tag="pA")
                for half in range(2):
                    m = mo + half
                    for j in range(NI // 2):
                        nc.tensor.matmul(pp[:, half, :],
                                         wfr[:, 2 * j:2 * j + 2,
                                             P * m:P * (m + 1)],
                                         fft2[c][j][:],
                                         start=(j == 0), stop=(j == NI // 2 - 1),
                                         perf_mode=PM.DoubleRow)
                    if flags["bf"]:
                        nc.tensor.matmul(pp[:, half, :],
                                         brow["bf"][:, P * m:P * (m + 1)],
                                         ones_row[:], start=False, stop=True,
                                         skip_group_check=True)
                nc.vector.scalar_tensor_tensor(
                    xT3[:, mo:mo + 2, cs], pp[:], cw["wf"] / SX,
                    xT2[:, mo:mo + 2, cs], op0=ALU.mult, op1=ALU.add)

        def store(c, cfunc=AF.Copy):
            xT3 = lazy["xT3"]
            for tp in range(2 * c, 2 * c + 2):
                pp = psA.tile([P, 2, NC], F32, tag="pA")
                for half in range(2):
                    stt = 2 * tp + half
                    for ft in range(FT):
                        nc.tensor.transpose(
                            pp[:, half, P * ft:P * (ft + 1)],
                            xT3[:, ft, P * stt:P * (stt + 1)], ident[:])
                for half in range(2):
                    ot = stage.tile([P, D], F32, tag="ot", bufs=3)
                    # ACT evac; func chosen to match the active act table
                    # (Identity = Ln/Exp table, Copy = gelu table)
                    nc.scalar.activation(ot[:], pp[:, half, :], cfunc)
                    nc.sync.dma_start(
                        out[P * (2 * tp + half):P * (2 * tp + half + 1), :],
                        ot[:])

        def dump(name, tile_ap):
            if dbg:
                nc.sync.dma_start(dbg_out[name][...], tile_ap)

        attn_self(qT1, kT1, v1sb, aT1, 0, [0], mid=emit_v_projs)
        for mo in range(0, FT, 2):
            project_pair(wq1r, "wq1", zv1, 1, mo, qT1, None,
                         "bq1", sd1.get(1), evac="dve")
        attn_self(qT1, kT1, v1sb, aT1, 0, [1, 2, 3])
        _enc_block()
        attn_self(qT1, kT1, v1sb, aT1, 1, [0, 1])
        pa1a(0)
        attn_self(qT1, kT1, v1sb, aT1, 1, [2])
        pa1b(0)
        attn_self(qT1, kT1, v1sb, aT1, 1, [3])
        pa1c(0)
        attn_cross(lazy["qT2"], 0, [0, 1])
        pa1a(1)
        attn_cross(lazy["qT2"], 0, [2, 3])
        pa1b(1)
        pa2a1(0)
        pa2a2(0)
        pa1c(1)
        # gelu c0 runs as one act-table-10 block; cross-attn c1 (table 6)
        # then overlaps the FF-c0 DVE/PE tail (ft8, wf matmuls, store)
        pa2b(0, [0, 1, 2, 3, 4, 5, 6, 7])
        attn_cross(lazy["qT2"], 1, [0, 1])
        attn_cross(lazy["qT2"], 1, [2, 3])
        pa2a1(1)
        ff_out(0)
        store(0, AF.Identity)
        pa2a2(1)
        pa2b(1, [0, 1, 2, 3, 4, 5, 6, 7])
        ff_out(1)
        store(1, AF.Copy)
        if dbg:
            dump("d_qT1", qT1[:])
            dump("d_kT1", kT1[:])
            dump("d_v1", v1sb[:])
            dump("d_aT1", aT1[:])
            dump("d_xT1", lazy["xT1"][:])
            dump("d_qT2", lazy["qT2"][:])
            dump("d_aT2", lazy["aT2"][:])
            dump("d_xT2", lazy["xT2"][:])
            dump("d_z3", lazy["z3"][:])
            dump("d_fft0", fft2[0][0][:])
            dump("d_kT2", kT2[:])
            dump("d_v2", v2sb[:])
            dump("d_encT", encT[:])

    return nc


_CACHED = {}


def _prep(inputs):
    """Host-side: fold LN gains, scale+convert weights to fp8, compute flags."""
    f32 = {k: np.ascontiguousarray(np.asarray(v), dtype=np.float32)
           for k, v in inputs.items()}
    g1, g2, g3 = f32["ln1_g"], f32["ln2_g"], f32["ln3_g"]
    b1, b2, b3 = f32["ln1_b"], f32["ln2_b"], f32["ln3_b"]
    wf32 = {
        "wq1": f32["wq1"] * g1[:, None], "wk1": f32["wk1"] * g1[:, None],
        "wv1": f32["wv1"] * g1[:, None], "wo1": f32["wo1"],
        "wq2": f32["wq2"] * g2[:, None], "wk2": f32["wk2"],
        "wv2": f32["wv2"], "wo2": f32["wo2"],
        "wg": f32["wg"] * g3[:, None], "wf": f32["wf"],
    }
    ws = {}
    w8 = {}
    for n, w in wf32.items():
        amax = float(np.abs(w).max()) or 1.0
        s = int(math.floor(math.log2(200.0 / amax)))
        ws[n] = s
        w8[n] = np.clip(w * (2.0 ** s), -240.0, 240.0).astype(
            ml_dtypes.float8_e4m3)
    bias = {
        "bq1": b1 @ f32["wq1"], "bk1": b1 @ f32["wk1"], "bv1": b1 @ f32["wv1"],
        "bq2": b2 @ f32["wq2"],
        "bo1": f32["bo1"], "bo2": f32["bo2"], "bf": f32["bf"],
        "bgu": b3 @ f32["wg"][:, :FF] + f32["bg"][:FF],
        "bgg": b3 @ f32["wg"][:, FF:] + f32["bg"][FF:],
    }
    flags = {n: bool(np.any(v != 0.0)) for n, v in bias.items()}
    dev_bias = {
        "bq1": bias["bq1"] * (SX * 2.0 ** ws["wq1"]),
        "bk1": bias["bk1"] * (SX * 2.0 ** ws["wk1"]),
        "bv1": bias["bv1"] * (SX * 2.0 ** ws["wv1"]),
        "bq2": bias["bq2"] * (SX * 2.0 ** ws["wq2"]),
        "bo1": bias["bo1"] * (SA * 2.0 ** ws["wo1"]),
        "bo2": bias["bo2"] * (SA * 2.0 ** ws["wo2"]),
        "bf": bias["bf"] * (SX * 2.0 ** ws["wf"]),
        "bgu": bias["bgu"] * (SX * 2.0 ** ws["wg"]),
        "bgg": bias["bgg"],    # true units (gelu bias)
    }
    return f32, w8, ws, dev_bias, flags


def _get_nc(key=None, ws=None, flags=None):
    if key is None:
        assert _CACHED, "kernel() must run before timeline queries"
        return next(iter(_CACHED.values()))
    if key not in _CACHED:
        nc = bacc.Bacc("TRN2", target_bir_lowering=False, debug=False,
                       num_devices=B)
        # bacc's act-table auto-pass maps Ln->natural_log and
        # Exp->exp_and_others (first table containing each func) and inserts
        # 15+ alternating table loads (~1.3us ACT each) across the
        # LN/softmax interleave. Feed it a doctored table list where only
        # natural_log_exp_and_others (Ln+Exp, one table for all of
        # layernorm+softmax) and gelu_and_others (Gelu+Copy, FF tail +
        # ACT store-copies) are matchable; emitted act_func_set_ids keep
        # their canonical positions so walrus' runtime remap is unchanged.
        import bass_rust as _bass_rust
        from concourse.hw_specs import get_activation_tables as _gat

        def _patched_insert_act_loads(_nc=nc):
            tabs = list(_gat(_nc.m.arch).items())
            doctored = []
            for name, funcs in tabs:
                if name == "natural_log_exp_and_others":
                    doctored.append((name, set(funcs) - {AF.Copy}))
                elif name == "gelu_and_others":
                    doctored.append((name, set(funcs)))
                else:
                    doctored.append((name, set()))
            _bass_rust.insert_act_table_loads(_nc, doctored)

        nc.insert_act_table_loads = _patched_insert_act_loads
        build(nc, ws, flags)
        nc.finalize()
        _CACHED[key] = nc
    return _CACHED[key]


def kernel(**inputs) -> np.ndarray:
    f32, w8, ws, dev_bias, flags = _prep(inputs)
    key = (tuple(sorted(flags.items())), tuple(sorted(ws.items())))
    nc = _get_nc(key, ws, flags)
    shared = dict(w8)
    for n, v in dev_bias.items():
        shared[n] = np.ascontiguousarray(v, dtype=np.float32)
    in_maps = [dict(shared, x=np.ascontiguousarray(f32["x"][i]),
                    enc=np.ascontiguousarray(f32["enc"][i])) for i in range(B)]
    res = run_bass_kernel_spmd(nc, in_maps, core_ids=list(range(B)))
    return np.stack([res.results[i]["out"] for i in range(B)], axis=0)


if __name__ == "__main__":
    print("module import ok")

